# revision 1
# baseline (speedup 1.0000x reference)
"""5-layer GAT (4x GATConv 128->128 heads=4, then GATConv 128->64 heads=1)
on 8 trn2 NeuronCores.

Sharding: edges partitioned by dst node across cores (each core owns 6272 dst
nodes = 49 blocks of 128). Per layer, a replicated node-feature table
[h | s_src] lives in shared DRAM, rebuilt each layer via AllGather of per-core
slices. Each core fetches h[src] rows for its edges with dma_gather (512B rows,
4 SWDGE queues, int16 indices over a lo/hi table split), computes edge softmax
numerators, and aggregates into per-block PSUM accumulators via one-hot
matmuls (edges pre-sorted by dst on the host, so each 128-edge tile belongs to
one 128-node block).
"""
import sys
sys.path.insert(0, '/opt/trn_rl_repo')

import numpy as np

import concourse.bass as bass
import concourse.bacc as bacc
import concourse.tile as tile
import concourse.mybir as mybir
from concourse.bass_utils import run_bass_kernel_spmd
from concourse.masks import make_identity

N = 50000
E = 1_600_000
IN = 128
HID = 32
HEADS = 4
HC = HEADS * HID          # 128
OUT = 64
NEG = 0.2

CORES = 8
NPC = 6272                # nodes per core
NB = CORES * NPC          # 50176
NBLK = NPC // 128         # 49
HALF = NB // 2            # 25088 == 4*NPC (int16-index table split)
TCOLS = 256               # bf16 table row: [h(128) | s_src(4) | pad] = 512B
TCOLS4 = 128              # fp32 table row: [h5(64) | s_src5(1) | pad] = 512B
MAXB = 4                  # tiles per gather batch (512 idxs)
NSWQ = 4

dt = mybir.dt
f32 = dt.float32
bf16 = dt.bfloat16

_cache = {}


def _rap(ap, free_dims):
    """Raw AP: keep partition dim of `ap`, replace free dims with [step,count] list."""
    return bass.AP(ap.tensor, ap.offset, [list(ap.ap[0])] + [list(d) for d in free_dims])


# ---------------------------------------------------------------- host prep

def _prep(edge_index):
    src = np.asarray(edge_index[0], dtype=np.int64)
    dst = np.asarray(edge_index[1], dtype=np.int64)

    core = dst // NPC
    blk = (dst % NPC) // 128
    dl_val = (dst % NPC) % 128
    half = (src >= HALF).astype(np.int64)

    key = (core * NBLK + blk) * 2 + half
    cnt = np.bincount(key, minlength=CORES * NBLK * 2).reshape(CORES, NBLK, 2)
    tiles_per = np.maximum(np.ceil(cnt / 128).astype(np.int64).max(axis=0), 1)  # [NBLK,2]
    T_LO, T_HI = tiles_per[:, 0], tiles_per[:, 1]
    TOT_TILES = int((T_LO + T_HI).sum())
    TOT_SLOTS = TOT_TILES * 128

    group_tiles = tiles_per.reshape(-1)                              # [NBLK*2]
    group_base = np.concatenate([[0], np.cumsum(group_tiles)[:-1]]) * 128

    # batch schedule: (block, half, nt, tile0, slot0); batches ordered by slot
    sched = []
    tcur = 0
    for b in range(NBLK):
        for h in range(2):
            ntiles = int(group_tiles[b * 2 + h])
            done = 0
            while done < ntiles:
                nt = min(MAXB, ntiles - done)
                sched.append((b, h, nt, tcur, int(group_base[b * 2 + h]) + done * 128))
                tcur += nt
                done += nt
    assert tcur == TOT_TILES
    NBATCH = len(sched)

    per_core = []
    for k in range(CORES):
        m = core == k
        s_k, blk_k, dl_k, half_k = src[m], blk[m], dl_val[m], half[m]
        gkey = blk_k * 2 + half_k
        order = np.argsort(gkey, kind='stable')
        s_k, dl_k, gkey = s_k[order], dl_k[order], gkey[order]
        gcnt = np.bincount(gkey, minlength=NBLK * 2)
        starts = np.concatenate([[0], np.cumsum(gcnt)[:-1]])
        rank = np.arange(len(gkey)) - starts[gkey]
        slot = group_base[gkey] + rank

        src_slot = np.zeros(TOT_SLOTS, dtype=np.int64)               # pad -> row 0
        dl_slot = np.full(TOT_SLOTS, -1.0, dtype=np.float32)         # pad -> -1
        src_slot[slot] = np.where(s_k >= HALF, s_k - HALF, s_k)
        dl_slot[slot] = dl_k.astype(np.float32)

        # wrapped int16 indices: per batch, idx i -> partition i%16, col i//16;
        # replicated into all 8 groups of 16 partitions
        seg_all = src_slot.astype(np.int16).reshape(TOT_SLOTS // 16, 16).T  # [16, S/16]
        idx16 = np.tile(seg_all, (8, 1))                             # [128, S/16]

        dl_arr = np.ascontiguousarray(dl_slot.reshape(TOT_TILES, 128).T)  # [128, T]

        dlrow = np.zeros((NBATCH, 512), dtype=np.float32)
        for i, (_b, _h, nt, _t0, slot0) in enumerate(sched):
            dlrow[i, 0:nt * 128] = dl_slot[slot0:slot0 + nt * 128]
        per_core.append((idx16, dl_arr, dlrow))

    return sched, T_LO, T_HI, TOT_TILES, NBATCH, per_core


def _prep_weights(W_stack, asrc_stack, adst_stack, b_stack,
                  W_last, asrc_last, adst_last, b_last):
    wcat = np.zeros((4, IN, 136), dtype=np.float32)
    for l in range(4):
        W = np.asarray(W_stack[l], dtype=np.float32)
        As = np.zeros((HC, HEADS), dtype=np.float32)
        Ad = np.zeros((HC, HEADS), dtype=np.float32)
        for h in range(HEADS):
            As[h * HID:(h + 1) * HID, h] = np.asarray(asrc_stack[l][h])
            Ad[h * HID:(h + 1) * HID, h] = np.asarray(adst_stack[l][h])
        wcat[l, :, :HC] = W
        wcat[l, :, HC:HC + HEADS] = W @ As
        wcat[l, :, HC + HEADS:] = W @ Ad
    WL = np.asarray(W_last, dtype=np.float32)
    wcat4 = np.zeros((HC, 66), dtype=np.float32)
    wcat4[:, :OUT] = WL
    wcat4[:, OUT] = WL @ np.asarray(asrc_last, dtype=np.float32)[0]
    wcat4[:, OUT + 1] = WL @ np.asarray(adst_last, dtype=np.float32)[0]
    bias = np.tile(np.asarray(b_stack, dtype=np.float32)[:, None, :], (1, 128, 1))
    bias4 = np.tile(np.asarray(b_last, dtype=np.float32)[None, :], (128, 1))
    return wcat, wcat4, bias, bias4


# ---------------------------------------------------------------- device program

def _build(sched, T_LO, T_HI, TOT_TILES, NBATCH):
    IDX_COLS = TOT_TILES * 8
    nc = bacc.Bacc("TRN2", target_bir_lowering=False, debug=False,
                   num_devices=CORES, num_swdge_queues=NSWQ)

    xs = nc.dram_tensor("xs", [NPC, IN], f32, kind="ExternalInput")
    idx16_in = nc.dram_tensor("idx16", [128, IDX_COLS], dt.int16, kind="ExternalInput")
    dl_in = nc.dram_tensor("dl", [128, TOT_TILES], f32, kind="ExternalInput")
    dlrow_in = nc.dram_tensor("dlrow", [NBATCH, 512], f32, kind="ExternalInput")
    wcat_in = nc.dram_tensor("wcat", [4, IN, 136], f32, kind="ExternalInput")
    wcat4_in = nc.dram_tensor("wcat4", [HC, 66], f32, kind="ExternalInput")
    bias_in = nc.dram_tensor("bias", [4, 128, 128], f32, kind="ExternalInput")
    bias4_in = nc.dram_tensor("bias4", [128, OUT], f32, kind="ExternalInput")
    out_ext = nc.dram_tensor("out", [NPC, OUT], f32, kind="ExternalOutput")
    import os
    KDEBUG = bool(int(os.environ.get("KDEBUG", "0")))
    if KDEBUG:
        dbg_gb = nc.dram_tensor("dbg_gb", [128, TCOLS], bf16, kind="ExternalOutput")
        dbg_sde = nc.dram_tensor("dbg_sde", [128, 16], f32, kind="ExternalOutput")
        dbg_sc = nc.dram_tensor("dbg_sc", [128, 16], f32, kind="ExternalOutput")
        dbg_ex = nc.dram_tensor("dbg_ex", [128, 16], f32, kind="ExternalOutput")
        dbg_msg = nc.dram_tensor("dbg_msg", [128, 132], bf16, kind="ExternalOutput")
        dbg_hb0 = nc.dram_tensor("dbg_hb0", [NPC, TCOLS], bf16, kind="ExternalOutput")
        dbg_tbl0 = nc.dram_tensor("dbg_tbl0", [NB, TCOLS], bf16, kind="ExternalOutput")
        dbg_act = nc.dram_tensor("dbg_act", [NBLK * 128, 128], f32, kind="ExternalOutput")
        dbg_denom = nc.dram_tensor("dbg_denom", [NBLK * 128, 4], f32, kind="ExternalOutput")

    tbl = [nc.dram_tensor(f"tbl{l}", [NB, TCOLS], bf16, kind="Internal",
                          addr_space="Shared") for l in range(4)]
    tbl4 = nc.dram_tensor("tbl4", [NB, TCOLS4], f32, kind="Internal",
                          addr_space="Shared")
    hb = [nc.dram_tensor(f"hb{l}", [NPC, TCOLS], bf16, kind="Internal")
          for l in range(4)]
    hb4 = nc.dram_tensor("hb4", [NPC, TCOLS4], f32, kind="Internal")

    RG = [list(range(CORES))]

    with tile.TileContext(nc) as tc:
        with tc.tile_pool(name="const", bufs=1) as cpool, \
             tc.tile_pool(name="work", bufs=3) as wpool, \
             tc.tile_pool(name="gbuf", bufs=4) as gpool, \
             tc.tile_pool(name="spool", bufs=10) as spool, \
             tc.tile_pool(name="psA", bufs=2, space="PSUM") as psA, \
             tc.tile_pool(name="psB", bufs=2, space="PSUM") as psB, \
             tc.tile_pool(name="psC", bufs=1, space="PSUM") as psC:

            # ---- constants
            iota_row_i = cpool.tile([128, 128], dt.int32)
            nc.gpsimd.iota(iota_row_i[:], pattern=[[1, 128]], base=0, channel_multiplier=0)
            iota_row = cpool.tile([128, 128], f32)
            nc.vector.tensor_copy(iota_row[:], iota_row_i[:])
            iota_col_i = cpool.tile([128, 1], dt.int32)
            nc.gpsimd.iota(iota_col_i[:], pattern=[[0, 1]], base=0, channel_multiplier=1)
            iota_col = cpool.tile([128, 1], f32)
            nc.vector.tensor_copy(iota_col[:], iota_col_i[:])
            ones_row = cpool.tile([1, 128], f32)
            nc.gpsimd.memset(ones_row[:], 1.0)
            ident = cpool.tile([128, 128], f32)
            make_identity(nc, ident[:])

            idx_sb = cpool.tile([128, IDX_COLS], dt.int16)
            nc.sync.dma_start(idx_sb[:], idx16_in[:])
            dl_sb = cpool.tile([128, TOT_TILES], f32)
            nc.sync.dma_start(dl_sb[:], dl_in[:])

            wcat_sb = cpool.tile([128, 4 * 136], bf16)
            for l in range(4):
                nc.gpsimd.dma_start(wcat_sb[:, l * 136:(l + 1) * 136], wcat_in[l])
            wcat4_sb = cpool.tile([128, 66], bf16)
            nc.gpsimd.dma_start(wcat4_sb[:], wcat4_in[:])
            bias_sb = cpool.tile([128, 4 * 128], f32)
            for l in range(4):
                nc.sync.dma_start(bias_sb[:, l * 128:(l + 1) * 128], bias_in[l])
            bias4_sb = cpool.tile([128, OUT], f32)
            nc.sync.dma_start(bias4_sb[:], bias4_in[:])

            sdst_sb = [cpool.tile([128, NBLK * 4], bf16, tag=f"sdst{i}",
                                  name=f"sdst{i}") for i in range(2)]
            sdst4_sb = cpool.tile([128, NBLK], f32)

            def node_phase(l, b, act_ap):
                """Project block-b activations into layer-l table staging + s_dst."""
                tp = psC.tile([128, 128], f32, tag="tp")
                nc.tensor.transpose(tp[:], act_ap, ident[:])
                actT = wpool.tile([128, 128], bf16, tag="actT")
                nc.vector.tensor_copy(actT[:], tp[:])
                if l < 4:
                    ntp = psC.tile([128, 136], f32, tag="ntp")
                    nc.tensor.matmul(ntp[:], lhsT=actT[:],
                                     rhs=wcat_sb[:, l * 136:(l + 1) * 136],
                                     start=True, stop=True)
                    stage = wpool.tile([128, 132], bf16, tag="stage")
                    nc.vector.tensor_copy(stage[:], ntp[:, 0:132])
                    nc.scalar.copy(sdst_sb[l % 2][:, 4 * b:4 * b + 4], ntp[:, 132:136])
                    nc.sync.dma_start(hb[l][b * 128:(b + 1) * 128, 0:132], stage[:])
                else:
                    ntp = psC.tile([128, 66], f32, tag="ntp")
                    nc.tensor.matmul(ntp[:], lhsT=actT[:], rhs=wcat4_sb[:],
                                     start=True, stop=True)
                    stage4 = wpool.tile([128, 65], f32, tag="stage4")
                    nc.vector.tensor_copy(stage4[:], ntp[:, 0:65])
                    nc.scalar.copy(sdst4_sb[:, b:b + 1], ntp[:, 65:66])
                    nc.sync.dma_start(hb4[b * 128:(b + 1) * 128, 0:65], stage4[:])

            # ---- layer 0 node phase: build table0 from xs
            for b in range(NBLK):
                xt = wpool.tile([128, 128], f32, tag="xt")
                nc.sync.dma_start(xt[:], xs[b * 128:(b + 1) * 128, :])
                node_phase(0, b, xt[:])
            nc.gpsimd.collective_compute("AllGather", mybir.AluOpType.bypass,
                                         replica_groups=RG, ins=[hb[0].ap().opt()],
                                         outs=[tbl[0].ap().opt()])
            if KDEBUG:
                nc.sync.dma_start(dbg_hb0[:], hb[0][:])
                nc.sync.dma_start(dbg_tbl0[:], tbl[0][:])

            # ---- per-block grouping of the batch schedule
            blocks = []
            for i, ent in enumerate(sched):
                if not blocks or ent[0] != blocks[-1][-1][1][0]:
                    blocks.append([])
                blocks[-1].append((i, ent))

            qrot = [0]

            def edge_layer(l):
                final = l == 4
                nh = 1 if final else HEADS
                ch = OUT if final else HID
                mc = nh * ch + nh                  # 65 or 132
                table = tbl4 if final else tbl[l]
                tdt = f32 if final else bf16
                elem = TCOLS4 if final else TCOLS
                scol = nh * ch                     # s_src col in table row
                sdst_cur = sdst4_sb if final else sdst_sb[l % 2]

                for batches in blocks:
                    b = batches[0][1][0]
                    ntiles_b = int(T_LO[b] + T_HI[b])
                    pblk = psA.tile([128, mc], f32, tag="pblk")
                    first = True
                    done_t = 0
                    for (bidx, (_b, hf, nt, t0, _slot0)) in batches:
                        G = nt * 128
                        gb = gpool.tile([128, MAXB, elem], tdt, tag="gb")
                        tin = table[HALF:NB, :] if hf else table[0:HALF, :]
                        nc.gpsimd.dma_gather(
                            out_ap=gb[:, 0:nt, :], in_ap=tin,
                            idxs_ap=idx_sb[:, t0 * 8:t0 * 8 + G // 16],
                            num_idxs=G, num_idxs_reg=G, elem_size=elem,
                            transpose=False, queue_num=qrot[0] % NSWQ)
                        qrot[0] += 1

                        dlr = wpool.tile([1, 512], f32, tag="dlr")
                        nc.sync.dma_start(dlr[0:1, 0:G], dlrow_in[bidx:bidx + 1, 0:G])
                        dlrep = psB.tile([128, 512], f32, tag="dlrep")
                        nc.tensor.matmul(
                            dlrep[:, 0:G], lhsT=ones_row[:],
                            rhs=dlr[0:1, 0:G],
                            start=True, stop=True)
                        sde = psB.tile([128, MAXB * 4], f32, tag="sde")
                        Ss = []
                        for j in range(nt):
                            S = spool.tile([128, 128], tdt, tag="S")
                            nc.vector.tensor_scalar(
                                out=S[:], in0=iota_row[:],
                                scalar1=dl_sb[:, t0 + j:t0 + j + 1],
                                scalar2=None, op0=mybir.AluOpType.is_equal)
                            Ss.append(S)
                            ST = spool.tile([128, 128], tdt, tag="ST")
                            nc.vector.tensor_scalar(
                                out=ST[:], in0=dlrep[:, j * 128:(j + 1) * 128],
                                scalar1=iota_col[:, 0:1],
                                scalar2=None, op0=mybir.AluOpType.is_equal)
                            nc.tensor.matmul(
                                sde[:, j * nh:(j + 1) * nh], lhsT=ST[:],
                                rhs=(sdst4_sb[:, b:b + 1] if final
                                     else sdst_sb[l % 2][:, 4 * b:4 * b + 4]),
                                start=True, stop=True)

                        # scores: sc[e, j*nh+h] = sde + gathered s_src
                        sc = wpool.tile([128, MAXB * 4], f32, tag="sc")
                        nc.vector.tensor_tensor(
                            out=_rap(sc[:], [[nh, nt], [1, nh]]),
                            in0=_rap(sde[:], [[nh, nt], [1, nh]]),
                            in1=gb[:, 0:nt, scol:scol + nh],
                            op=mybir.AluOpType.add)
                        sc2 = wpool.tile([128, MAXB * 4], f32, tag="sc2")
                        nc.scalar.mul(sc2[:, 0:nt * nh], sc[:, 0:nt * nh], NEG)
                        nc.vector.tensor_tensor(out=sc[:, 0:nt * nh], in0=sc[:, 0:nt * nh],
                                                in1=sc2[:, 0:nt * nh], op=mybir.AluOpType.max)
                        ex = wpool.tile([128, MAXB * 4], f32, tag="ex")
                        nc.scalar.activation(ex[:, 0:nt * nh], sc[:, 0:nt * nh],
                                             mybir.ActivationFunctionType.Exp)

                        msg = gpool.tile([128, MAXB, mc], tdt, tag="msg")
                        # ex columns into msg[:, j, nh*ch:nh*ch+nh]
                        nc.vector.tensor_copy(
                            msg[:, 0:nt, nh * ch:nh * ch + nh],
                            _rap(ex[:], [[nh, nt], [1, nh]]))
                        # msg[:, j, h*ch:(h+1)*ch] = gb * ex (per-partition scalar)
                        for j in range(nt):
                            for h in range(nh):
                                c0 = h * ch
                                nc.vector.tensor_scalar(
                                    out=msg[:, j, c0:c0 + ch],
                                    in0=gb[:, j, c0:c0 + ch],
                                    scalar1=ex[:, j * nh + h:j * nh + h + 1],
                                    scalar2=None, op0=mybir.AluOpType.mult)

                        if KDEBUG and l == 0 and bidx == 0:
                            nc.sync.dma_start(dbg_gb[:], gb[:, 0, :])
                            sdesb = wpool.tile([128, 16], f32, tag="sdesb")
                            nc.vector.tensor_copy(sdesb[:], sde[:])
                            nc.sync.dma_start(dbg_sde[:], sdesb[:])
                            nc.sync.dma_start(dbg_sc[:], sc[:])
                            nc.sync.dma_start(dbg_ex[:], ex[:])
                            nc.sync.dma_start(dbg_msg[:], msg[:, 0, :])
                        for j in range(nt):
                            nc.tensor.matmul(pblk[:], lhsT=Ss[j][:], rhs=msg[:, j, :],
                                             start=first,
                                             stop=(done_t + j == ntiles_b - 1))
                            first = False
                        done_t += nt

                    # ---- block epilogue
                    rec = wpool.tile([128, 4], f32, tag="rec")
                    nc.vector.tensor_scalar(out=rec[:, 0:nh], in0=pblk[:, nh * ch:nh * ch + nh],
                                            scalar1=1e-16, scalar2=None,
                                            op0=mybir.AluOpType.add)
                    nc.vector.reciprocal(rec[:, 0:nh], rec[:, 0:nh])
                    act = wpool.tile([128, 128], f32, tag="act")
                    for h in range(nh):
                        nc.vector.tensor_scalar(
                            out=act[:, h * ch:(h + 1) * ch],
                            in0=pblk[:, h * ch:(h + 1) * ch],
                            scalar1=rec[:, h:h + 1],
                            scalar2=None, op0=mybir.AluOpType.mult)
                    if KDEBUG and l == 0:
                        dnsb = wpool.tile([128, 4], f32, tag="dnsb")
                        nc.vector.tensor_copy(dnsb[:], pblk[:, nh * ch:nh * ch + 4])
                        nc.sync.dma_start(dbg_denom[b * 128:(b + 1) * 128, :], dnsb[:])
                    if final:
                        nc.vector.tensor_tensor(out=act[:, 0:OUT], in0=act[:, 0:OUT],
                                                in1=bias4_sb[:], op=mybir.AluOpType.add)
                        nc.sync.dma_start(out_ext[b * 128:(b + 1) * 128, :], act[:, 0:OUT])
                    else:
                        nc.vector.tensor_tensor(out=act[:], in0=act[:],
                                                in1=bias_sb[:, l * 128:(l + 1) * 128],
                                                op=mybir.AluOpType.add)
                        neg = wpool.tile([128, 128], f32, tag="neg")
                        nc.vector.tensor_scalar(out=neg[:], in0=act[:], scalar1=0.0,
                                                scalar2=None, op0=mybir.AluOpType.min)
                        en = wpool.tile([128, 128], f32, tag="en")
                        nc.scalar.activation(en[:], neg[:], mybir.ActivationFunctionType.Exp)
                        pos = wpool.tile([128, 128], f32, tag="pos")
                        nc.scalar.activation(pos[:], act[:], mybir.ActivationFunctionType.Relu)
                        nc.vector.tensor_tensor(out=act[:], in0=en[:], in1=pos[:],
                                                op=mybir.AluOpType.add)
                        nc.vector.tensor_scalar(out=act[:], in0=act[:], scalar1=-1.0,
                                                scalar2=None, op0=mybir.AluOpType.add)
                        if KDEBUG and l == 0:
                            nc.sync.dma_start(dbg_act[b * 128:(b + 1) * 128, :], act[:])
                        node_phase(l + 1, b, act[:])

                if not final:
                    lp = l + 1
                    src_hb = hb[lp].ap().opt() if lp < 4 else hb4.ap().opt()
                    dst_tbl = tbl[lp].ap().opt() if lp < 4 else tbl4.ap().opt()
                    nc.gpsimd.collective_compute("AllGather", mybir.AluOpType.bypass,
                                                 replica_groups=RG,
                                                 ins=[src_hb], outs=[dst_tbl])

            for l in range(5):
                edge_layer(l)

    nc.compile()
    return nc


# ---------------------------------------------------------------- entry point

def kernel(x, edge_index, W_stack, asrc_stack, adst_stack, b_stack,
           W_last, asrc_last, adst_last, b_last):
    ek = np.asarray(edge_index)
    ckb = ek.tobytes()
    ck = (len(ckb), ckb[:512], ckb[-512:])
    if 'nc' not in _cache or _cache.get('ck') != ck:
        sched, T_LO, T_HI, TOT_TILES, NBATCH, per_core = _prep(ek)
        nc = _build(sched, T_LO, T_HI, TOT_TILES, NBATCH)
        _cache.update(nc=nc, per_core=per_core, ck=ck)
    nc = _cache['nc']
    per_core = _cache['per_core']

    wcat, wcat4, bias, bias4 = _prep_weights(
        W_stack, asrc_stack, adst_stack, b_stack,
        W_last, asrc_last, adst_last, b_last)

    x_np = np.asarray(x, dtype=np.float32)
    xs_pad = np.zeros((NB, IN), dtype=np.float32)
    xs_pad[:N] = x_np

    in_maps = []
    for k in range(CORES):
        idx16, dl_arr, dlrow = per_core[k]
        in_maps.append({
            "xs": np.ascontiguousarray(xs_pad[k * NPC:(k + 1) * NPC]),
            "idx16": idx16, "dl": dl_arr, "dlrow": dlrow,
            "wcat": wcat, "wcat4": wcat4, "bias": bias, "bias4": bias4,
        })
    res = run_bass_kernel_spmd(nc, in_maps, core_ids=list(range(CORES)))
    import os
    if int(os.environ.get("KDEBUG", "0")):
        _cache['dbg'] = res.results
    out = np.concatenate([res.results[k]["out"] for k in range(CORES)], axis=0)
    return out[:N].astype(np.float32)



# revision 5
# speedup vs baseline: 1979.6418x; 1979.6418x over previous
"""5-layer GAT (4x GATConv 128->128 heads=4, then GATConv 128->64 heads=1)
on 8 trn2 NeuronCores.

Sharding: edges partitioned by dst node across cores (each core owns 6272 dst
nodes = 49 blocks of 128). Per layer, a replicated node-feature table
[h | s_src] lives in shared DRAM, rebuilt each layer via AllGather of per-core
slices. Each core fetches h[src] rows for its edges with dma_gather (512B rows
layers 0-3, 256B bf16 rows layer 4; 4 SWDGE queues, int16 indices over a
lo/hi table split), computes edge softmax numerators, and aggregates into
per-block PSUM accumulators via one-hot matmuls (edges pre-sorted by dst on
the host, so each 128-edge tile belongs to one 128-node block).

The edge pipeline is fused: per batch of up to 8 tiles (1024 edges) it does
one tensor_tensor is_equal for all slot->node one-hots (S4), PE transposes +
one PSUM->SBUF copy for the node->slot one-hots (ST), per-tile sde matmuls,
one add, Prelu+Exp on the scalar engine, and a single broadcast multiply for
all messages.

Execution: a persistent jitted PJRT executable is cached along with
device-resident edge-derived constant inputs; per call only x and the small
weights are re-shipped.
"""
import sys
sys.path.insert(0, '/opt/trn_rl_repo')

import numpy as np

import concourse.bass as bass
import concourse.bacc as bacc
import concourse.tile as tile
import concourse.mybir as mybir
from concourse.masks import make_identity

N = 50000
E = 1_600_000
IN = 128
HID = 32
HEADS = 4
HC = HEADS * HID          # 128
OUT = 64
NEG = 0.2

CORES = 8
NPC = 6272                # nodes per core
NB = CORES * NPC          # 50176
NBLK = NPC // 128         # 49
HALF = NB // 2            # 25088 == 4*NPC (int16-index table split)
TCOLS = 256               # bf16 table row: [h(128) | s_src(4) | pad] = 512B
TCOLS4 = 128              # bf16 table row: [h5(64) | s_src5(1) | pad] = 256B
MAXB = 8                  # tiles per gather batch (1024 idxs)
NSWQ = 4

dt = mybir.dt
f32 = dt.float32
bf16 = dt.bfloat16

_cache = {}


def _rap(ap, free_dims):
    """Raw AP: keep partition dim of `ap`, replace free dims with [step,count] list."""
    return bass.AP(ap.tensor, ap.offset, [list(ap.ap[0])] + [list(d) for d in free_dims])


# ---------------------------------------------------------------- host prep

def _prep(edge_index):
    src = np.asarray(edge_index[0], dtype=np.int64)
    dst = np.asarray(edge_index[1], dtype=np.int64)

    core = dst // NPC
    blk = (dst % NPC) // 128
    dl_val = (dst % NPC) % 128
    half = (src >= HALF).astype(np.int64)

    key = (core * NBLK + blk) * 2 + half
    cnt = np.bincount(key, minlength=CORES * NBLK * 2).reshape(CORES, NBLK, 2)
    tiles_per = np.maximum(np.ceil(cnt / 128).astype(np.int64).max(axis=0), 1)  # [NBLK,2]
    T_LO, T_HI = tiles_per[:, 0], tiles_per[:, 1]
    TOT_TILES = int((T_LO + T_HI).sum())
    TOT_SLOTS = TOT_TILES * 128

    group_tiles = tiles_per.reshape(-1)                              # [NBLK*2]
    group_base = np.concatenate([[0], np.cumsum(group_tiles)[:-1]]) * 128

    # batch schedule: (block, half, nt, tile0, slot0); batches ordered by slot
    sched = []
    tcur = 0
    for b in range(NBLK):
        for h in range(2):
            ntiles = int(group_tiles[b * 2 + h])
            done = 0
            while done < ntiles:
                nt = min(MAXB, ntiles - done)
                sched.append((b, h, nt, tcur, int(group_base[b * 2 + h]) + done * 128))
                tcur += nt
                done += nt
    assert tcur == TOT_TILES
    NBATCH = len(sched)

    per_core = []
    for k in range(CORES):
        m = core == k
        s_k, blk_k, dl_k, half_k = src[m], blk[m], dl_val[m], half[m]
        gkey = blk_k * 2 + half_k
        order = np.argsort(gkey, kind='stable')
        s_k, dl_k, gkey = s_k[order], dl_k[order], gkey[order]
        gcnt = np.bincount(gkey, minlength=NBLK * 2)
        starts = np.concatenate([[0], np.cumsum(gcnt)[:-1]])
        rank = np.arange(len(gkey)) - starts[gkey]
        slot = group_base[gkey] + rank

        src_slot = np.zeros(TOT_SLOTS, dtype=np.int64)               # pad -> row 0
        dl_slot = np.full(TOT_SLOTS, -1.0, dtype=np.float32)         # pad -> -1
        src_slot[slot] = np.where(s_k >= HALF, s_k - HALF, s_k)
        dl_slot[slot] = dl_k.astype(np.float32)

        # wrapped int16 indices: per batch, idx i -> partition i%16, col i//16;
        # replicated into all 8 groups of 16 partitions
        seg_all = src_slot.astype(np.int16).reshape(TOT_SLOTS // 16, 16).T  # [16, S/16]
        idx16 = np.tile(seg_all, (8, 1))                             # [128, S/16]

        dl_arr = np.ascontiguousarray(dl_slot.reshape(TOT_TILES, 128).T)  # [128, T]
        per_core.append((idx16, dl_arr))

    return sched, T_LO, T_HI, TOT_TILES, NBATCH, per_core


def _prep_weights(W_stack, asrc_stack, adst_stack, b_stack,
                  W_last, asrc_last, adst_last, b_last):
    wcat = np.zeros((4, IN, 136), dtype=np.float32)
    for l in range(4):
        W = np.asarray(W_stack[l], dtype=np.float32)
        As = np.zeros((HC, HEADS), dtype=np.float32)
        Ad = np.zeros((HC, HEADS), dtype=np.float32)
        for h in range(HEADS):
            As[h * HID:(h + 1) * HID, h] = np.asarray(asrc_stack[l][h])
            Ad[h * HID:(h + 1) * HID, h] = np.asarray(adst_stack[l][h])
        wcat[l, :, :HC] = W
        wcat[l, :, HC:HC + HEADS] = W @ As
        wcat[l, :, HC + HEADS:] = W @ Ad
    WL = np.asarray(W_last, dtype=np.float32)
    wcat4 = np.zeros((HC, 66), dtype=np.float32)
    wcat4[:, :OUT] = WL
    wcat4[:, OUT] = WL @ np.asarray(asrc_last, dtype=np.float32)[0]
    wcat4[:, OUT + 1] = WL @ np.asarray(adst_last, dtype=np.float32)[0]
    bias = np.tile(np.asarray(b_stack, dtype=np.float32)[:, None, :], (1, 128, 1))
    bias4 = np.tile(np.asarray(b_last, dtype=np.float32)[None, :], (128, 1))
    return wcat, wcat4, bias, bias4


# ---------------------------------------------------------------- device program

def _build(sched, T_LO, T_HI, TOT_TILES, NBATCH):
    IDX_COLS = TOT_TILES * 8
    nc = bacc.Bacc("TRN2", target_bir_lowering=False, debug=False,
                   num_devices=CORES, num_swdge_queues=NSWQ)

    xs = nc.dram_tensor("xs", [NPC, IN], f32, kind="ExternalInput")
    idx16_in = nc.dram_tensor("idx16", [128, IDX_COLS], dt.int16, kind="ExternalInput")
    dl_in = nc.dram_tensor("dl", [128, TOT_TILES], f32, kind="ExternalInput")
    wcat_in = nc.dram_tensor("wcat", [4, IN, 136], f32, kind="ExternalInput")
    wcat4_in = nc.dram_tensor("wcat4", [HC, 66], f32, kind="ExternalInput")
    bias_in = nc.dram_tensor("bias", [4, 128, 128], f32, kind="ExternalInput")
    bias4_in = nc.dram_tensor("bias4", [128, OUT], f32, kind="ExternalInput")
    out_ext = nc.dram_tensor("out", [NPC, OUT], f32, kind="ExternalOutput")

    tbl = [nc.dram_tensor(f"tbl{l}", [NB, TCOLS], bf16, kind="Internal",
                          addr_space="Shared") for l in range(4)]
    tbl4 = nc.dram_tensor("tbl4", [NB, TCOLS4], bf16, kind="Internal",
                          addr_space="Shared")
    hb = [nc.dram_tensor(f"hb{l}", [NPC, TCOLS], bf16, kind="Internal")
          for l in range(4)]
    hb4 = nc.dram_tensor("hb4", [NPC, TCOLS4], bf16, kind="Internal")

    RG = [list(range(CORES))]

    with tile.TileContext(nc) as tc:
        with tc.tile_pool(name="const", bufs=1) as cpool, \
             tc.tile_pool(name="work", bufs=3) as wpool, \
             tc.tile_pool(name="gbuf", bufs=4) as gpool, \
             tc.tile_pool(name="spool", bufs=4) as spool, \
             tc.tile_pool(name="psA", bufs=2, space="PSUM") as psA, \
             tc.tile_pool(name="psB", bufs=2, space="PSUM") as psB, \
             tc.tile_pool(name="psC", bufs=1, space="PSUM") as psC:

            # ---- constants
            iota_row_i = cpool.tile([128, 128], dt.int32)
            nc.gpsimd.iota(iota_row_i[:], pattern=[[1, 128]], base=0, channel_multiplier=0)
            iota_row = cpool.tile([128, 128], bf16)
            nc.vector.tensor_copy(iota_row[:], iota_row_i[:])
            ident = cpool.tile([128, 128], f32)
            make_identity(nc, ident[:])
            ident_bf = cpool.tile([128, 128], bf16)
            nc.vector.tensor_copy(ident_bf[:], ident[:])

            idx_sb = cpool.tile([128, IDX_COLS], dt.int16)
            nc.sync.dma_start(idx_sb[:], idx16_in[:])
            dl_f = cpool.tile([128, TOT_TILES], f32)
            nc.sync.dma_start(dl_f[:], dl_in[:])
            dl_sb = cpool.tile([128, TOT_TILES], bf16)
            nc.vector.tensor_copy(dl_sb[:], dl_f[:])

            wcat_sb = cpool.tile([128, 4 * 136], bf16)
            for l in range(4):
                nc.gpsimd.dma_start(wcat_sb[:, l * 136:(l + 1) * 136], wcat_in[l])
            wcat4_sb = cpool.tile([128, 66], bf16)
            nc.gpsimd.dma_start(wcat4_sb[:], wcat4_in[:])
            bias_sb = cpool.tile([128, 4 * 128], f32)
            for l in range(4):
                nc.sync.dma_start(bias_sb[:, l * 128:(l + 1) * 128], bias_in[l])
            bias4_sb = cpool.tile([128, OUT], f32)
            nc.sync.dma_start(bias4_sb[:], bias4_in[:])

            sdst_sb = [cpool.tile([128, NBLK * 4], bf16, tag=f"sdst{i}",
                                  name=f"sdst{i}") for i in range(2)]
            sdst4_sb = cpool.tile([128, NBLK], bf16)

            def node_phase(l, b, act_ap):
                """Project block-b activations into layer-l table staging + s_dst."""
                tp = psC.tile([128, 128], f32, tag="tp")
                nc.tensor.transpose(tp[:], act_ap, ident[:])
                actT = wpool.tile([128, 128], bf16, tag="actT")
                nc.vector.tensor_copy(actT[:], tp[:])
                if l < 4:
                    ntp = psC.tile([128, 136], f32, tag="ntp")
                    nc.tensor.matmul(ntp[:], lhsT=actT[:],
                                     rhs=wcat_sb[:, l * 136:(l + 1) * 136],
                                     start=True, stop=True)
                    stage = wpool.tile([128, 132], bf16, tag="stage")
                    nc.vector.tensor_copy(stage[:], ntp[:, 0:132])
                    nc.scalar.copy(sdst_sb[l % 2][:, 4 * b:4 * b + 4], ntp[:, 132:136])
                    nc.sync.dma_start(hb[l][b * 128:(b + 1) * 128, 0:132], stage[:])
                else:
                    ntp = psC.tile([128, 66], f32, tag="ntp")
                    nc.tensor.matmul(ntp[:], lhsT=actT[:], rhs=wcat4_sb[:],
                                     start=True, stop=True)
                    stage4 = wpool.tile([128, 65], bf16, tag="stage4")
                    nc.vector.tensor_copy(stage4[:], ntp[:, 0:65])
                    nc.scalar.copy(sdst4_sb[:, b:b + 1], ntp[:, 65:66])
                    nc.sync.dma_start(hb4[b * 128:(b + 1) * 128, 0:65], stage4[:])

            # ---- layer 0 node phase: build table0 from xs
            for b in range(NBLK):
                xt = wpool.tile([128, 128], f32, tag="xt")
                nc.sync.dma_start(xt[:], xs[b * 128:(b + 1) * 128, :])
                node_phase(0, b, xt[:])
            nc.gpsimd.collective_compute("AllGather", mybir.AluOpType.bypass,
                                         replica_groups=RG, ins=[hb[0].ap().opt()],
                                         outs=[tbl[0].ap().opt()])

            # ---- per-block grouping of the batch schedule
            blocks = []
            for i, ent in enumerate(sched):
                if not blocks or ent[0] != blocks[-1][-1][1][0]:
                    blocks.append([])
                blocks[-1].append((i, ent))

            qrot = [0]

            def edge_layer(l):
                final = l == 4
                nh = 1 if final else HEADS
                ch = OUT if final else HID
                mc = nh * ch + nh                  # 65 or 132
                table = tbl4 if final else tbl[l]
                elem = TCOLS4 if final else TCOLS
                scol = nh * ch                     # s_src col in table row

                for batches in blocks:
                    b = batches[0][1][0]
                    ntiles_b = int(T_LO[b] + T_HI[b])
                    pblk = psA.tile([128, mc], f32, tag="pblk")
                    first = True
                    done_t = 0
                    for (bidx, (_b, hf, nt, t0, _slot0)) in batches:
                        G = nt * 128
                        gb = gpool.tile([128, MAXB * TCOLS], bf16, tag="gb")
                        tin = table[HALF:NB, :] if hf else table[0:HALF, :]
                        nc.gpsimd.dma_gather(
                            out_ap=_rap(gb[:], [[elem, nt], [1, elem]]),
                            in_ap=tin,
                            idxs_ap=idx_sb[:, t0 * 8:t0 * 8 + G // 16],
                            num_idxs=G, num_idxs_reg=G, elem_size=elem,
                            transpose=False, queue_num=qrot[0] % NSWQ)
                        qrot[0] += 1

                        # S4[p=slot, (j, node)] = (node == dl[slot of tile j])
                        S4 = spool.tile([128, MAXB * 128], bf16, tag="S4")
                        nc.vector.tensor_tensor(
                            out=_rap(S4[:], [[128, nt], [1, 128]]),
                            in0=_rap(iota_row[:], [[0, nt], [1, 128]]),
                            in1=_rap(dl_sb[:, t0:t0 + 1], [[1, nt], [0, 128]]),
                            op=mybir.AluOpType.is_equal)
                        # ST = per-tile transpose of S4 (node -> slot one-hot)
                        STp = psB.tile([128, MAXB * 128], bf16, tag="STp")
                        for j in range(nt):
                            nc.tensor.transpose(STp[:, j * 128:(j + 1) * 128],
                                                S4[:, j * 128:(j + 1) * 128],
                                                ident_bf[:])
                        ST = spool.tile([128, MAXB * 128], bf16, tag="ST")
                        nc.vector.tensor_copy(ST[:, 0:nt * 128], STp[:, 0:nt * 128])
                        # sde[slot, (j,h)] = s_dst[dl[slot], h]
                        sde = psB.tile([128, MAXB * 4], f32, tag="sde")
                        for j in range(nt):
                            nc.tensor.matmul(
                                sde[:, j * nh:(j + 1) * nh],
                                lhsT=ST[:, j * 128:(j + 1) * 128],
                                rhs=(sdst4_sb[:, b:b + 1] if final
                                     else sdst_sb[l % 2][:, 4 * b:4 * b + 4]),
                                start=True, stop=True)

                        # scores: sc = sde + gathered s_src
                        sc = wpool.tile([128, MAXB * 4], f32, tag="sc")
                        nc.vector.tensor_tensor(
                            out=_rap(sc[:], [[nh, nt], [1, nh]]),
                            in0=_rap(sde[:], [[nh, nt], [1, nh]]),
                            in1=_rap(gb[:, scol:scol + 1], [[elem, nt], [1, nh]]),
                            op=mybir.AluOpType.add)
                        # leaky relu + exp on the scalar engine; exp lands in
                        # the msg numerator columns directly
                        lr = wpool.tile([128, MAXB * 4], f32, tag="lr")
                        nc.scalar.activation(lr[:, 0:nt * nh], sc[:, 0:nt * nh],
                                             mybir.ActivationFunctionType.Prelu,
                                             alpha=NEG)
                        msg = gpool.tile([128, MAXB * mc], bf16, tag="msg")
                        nc.scalar.activation(
                            _rap(msg[:, scol:scol + 1], [[mc, nt], [1, nh]]),
                            lr[:, 0:nt * nh],
                            mybir.ActivationFunctionType.Exp)
                        # msg[:, j, h*ch:(h+1)*ch] = gb * ex, one broadcast op
                        nc.vector.tensor_tensor(
                            out=_rap(msg[:], [[mc, nt], [ch, nh], [1, ch]]),
                            in0=_rap(gb[:], [[elem, nt], [ch, nh], [1, ch]]),
                            in1=_rap(msg[:, scol:scol + 1], [[mc, nt], [1, nh], [0, ch]]),
                            op=mybir.AluOpType.mult)

                        for j in range(nt):
                            nc.tensor.matmul(
                                pblk[:],
                                lhsT=S4[:, j * 128:(j + 1) * 128],
                                rhs=_rap(msg[:, j * mc:j * mc + 1], [[1, mc]]),
                                start=first,
                                stop=(done_t + j == ntiles_b - 1))
                            first = False
                        done_t += nt

                    # ---- block epilogue
                    rec = wpool.tile([128, 4], f32, tag="rec")
                    nc.vector.tensor_scalar(out=rec[:, 0:nh], in0=pblk[:, nh * ch:nh * ch + nh],
                                            scalar1=1e-16, scalar2=None,
                                            op0=mybir.AluOpType.add)
                    nc.vector.reciprocal(rec[:, 0:nh], rec[:, 0:nh])
                    act = wpool.tile([128, 128], f32, tag="act")
                    nc.vector.tensor_tensor(
                        out=_rap(act[:], [[ch, nh], [1, ch]]),
                        in0=_rap(pblk[:], [[ch, nh], [1, ch]]),
                        in1=_rap(rec[:], [[1, nh], [0, ch]]),
                        op=mybir.AluOpType.mult)
                    if final:
                        nc.vector.tensor_tensor(out=act[:, 0:OUT], in0=act[:, 0:OUT],
                                                in1=bias4_sb[:], op=mybir.AluOpType.add)
                        nc.sync.dma_start(out_ext[b * 128:(b + 1) * 128, :], act[:, 0:OUT])
                    else:
                        nc.vector.tensor_tensor(out=act[:], in0=act[:],
                                                in1=bias_sb[:, l * 128:(l + 1) * 128],
                                                op=mybir.AluOpType.add)
                        neg = wpool.tile([128, 128], f32, tag="neg")
                        nc.vector.tensor_scalar(out=neg[:], in0=act[:], scalar1=0.0,
                                                scalar2=None, op0=mybir.AluOpType.min)
                        en = wpool.tile([128, 128], f32, tag="en")
                        nc.scalar.activation(en[:], neg[:], mybir.ActivationFunctionType.Exp)
                        pos = wpool.tile([128, 128], f32, tag="pos")
                        nc.scalar.activation(pos[:], act[:], mybir.ActivationFunctionType.Relu)
                        nc.vector.tensor_tensor(out=act[:], in0=en[:], in1=pos[:],
                                                op=mybir.AluOpType.add)
                        nc.vector.tensor_scalar(out=act[:], in0=act[:], scalar1=-1.0,
                                                scalar2=None, op0=mybir.AluOpType.add)
                        node_phase(l + 1, b, act[:])

                if not final:
                    lp = l + 1
                    src_hb = hb[lp].ap().opt() if lp < 4 else hb4.ap().opt()
                    dst_tbl = tbl[lp].ap().opt() if lp < 4 else tbl4.ap().opt()
                    nc.gpsimd.collective_compute("AllGather", mybir.AluOpType.bypass,
                                                 replica_groups=RG,
                                                 ins=[src_hb], outs=[dst_tbl])

            for l in range(5):
                edge_layer(l)

    nc.compile()
    return nc


# ---------------------------------------------------------------- executor

def _make_exec(nc):
    """Build a persistent jitted PJRT executable for nc (multi-core SPMD)."""
    import jax
    from jax.sharding import Mesh, PartitionSpec, NamedSharding
    import warnings
    with warnings.catch_warnings():
        warnings.simplefilter("ignore")
        from jax.experimental.shard_map import shard_map
    from concourse.bass2jax import (_bass_exec_p, install_neuronx_cc_hook,
                                    partition_id_tensor)

    install_neuronx_cc_hook()
    partition_name = nc.partition_id_tensor.name if nc.partition_id_tensor else None
    in_names, out_names, out_avals, zero_outs = [], [], [], []
    for alloc in nc.m.functions[0].allocations:
        if not isinstance(alloc, mybir.MemoryLocationSet):
            continue
        name = alloc.memorylocations[0].name
        if alloc.kind == "ExternalInput":
            if name != partition_name:
                in_names.append(name)
        elif alloc.kind == "ExternalOutput":
            out_names.append(name)
            shape = tuple(alloc.tensor_shape)
            dtype = mybir.dt.np(alloc.dtype)
            out_avals.append(jax.core.ShapedArray(shape, dtype))
            zero_outs.append(np.zeros(shape, dtype))
    n_params = len(in_names)
    n_outs = len(out_avals)
    in_names_all = in_names + out_names + ([partition_name] if partition_name else [])

    def _body(*args):
        operands = list(args)
        if partition_name is not None:
            operands.append(partition_id_tensor())
        outs = _bass_exec_p.bind(
            *operands, out_avals=tuple(out_avals),
            in_names=tuple(in_names_all), out_names=tuple(out_names),
            lowering_input_output_aliases=(), sim_require_finite=True,
            sim_require_nnan=True, nc=nc)
        return tuple(outs)

    devices = jax.devices()[:CORES]
    mesh = Mesh(np.asarray(devices), ("core",))
    sh = NamedSharding(mesh, PartitionSpec("core"))
    in_specs = (PartitionSpec("core"),) * (n_params + n_outs)
    out_specs = (PartitionSpec("core"),) * n_outs
    run = jax.jit(
        shard_map(_body, mesh=mesh, in_specs=in_specs, out_specs=out_specs,
                  check_rep=False),
        keep_unused=True)
    dev_zeros = jax.device_put(
        [np.zeros((CORES * z.shape[0], *z.shape[1:]), z.dtype) for z in zero_outs],
        [sh] * n_outs)
    return dict(run=run, sharding=sh, in_names=in_names, out_names=out_names,
                dev_zeros=dev_zeros, jax=jax)


# ---------------------------------------------------------------- entry point

def kernel(x, edge_index, W_stack, asrc_stack, adst_stack, b_stack,
           W_last, asrc_last, adst_last, b_last):
    ek = np.asarray(edge_index)
    ckb = ek.tobytes()
    ck = (len(ckb), ckb[:512], ckb[-512:])
    if 'exec' not in _cache or _cache.get('ck') != ck:
        sched, T_LO, T_HI, TOT_TILES, NBATCH, per_core = _prep(ek)
        nc = _build(sched, T_LO, T_HI, TOT_TILES, NBATCH)
        ex = _make_exec(nc)
        jax = ex['jax']
        # edge-derived constants live on device across calls
        const_dev = {
            "idx16": jax.device_put(
                np.concatenate([per_core[k][0] for k in range(CORES)], axis=0),
                ex['sharding']),
            "dl": jax.device_put(
                np.concatenate([per_core[k][1] for k in range(CORES)], axis=0),
                ex['sharding']),
        }
        jax.block_until_ready(list(const_dev.values()))
        ex['const_dev'] = const_dev
        _cache.update(**{'exec': ex, 'ck': ck})
    ex = _cache['exec']
    jax = ex['jax']

    wcat, wcat4, bias, bias4 = _prep_weights(
        W_stack, asrc_stack, adst_stack, b_stack,
        W_last, asrc_last, adst_last, b_last)

    x_np = np.asarray(x, dtype=np.float32)
    xs_pad = np.zeros((NB, IN), dtype=np.float32)
    xs_pad[:N] = x_np

    host_in = {
        "xs": xs_pad,
        "wcat": np.tile(wcat, (CORES, 1, 1)),
        "wcat4": np.tile(wcat4, (CORES, 1)),
        "bias": np.tile(bias, (CORES, 1, 1)),
        "bias4": np.tile(bias4, (CORES, 1)),
    }
    dev_in = []
    for name in ex['in_names']:
        if name in ex['const_dev']:
            dev_in.append(ex['const_dev'][name])
        else:
            dev_in.append(jax.device_put(host_in[name], ex['sharding']))
    outs = ex['run'](*dev_in, *ex['dev_zeros'])
    _cache['last_dev_in'] = dev_in
    out_idx = ex['out_names'].index("out")
    out = np.asarray(outs[out_idx]).reshape(CORES, NPC, OUT).reshape(CORES * NPC, OUT)
    return out[:N].astype(np.float32)


def bench_exec(iters=24, warmup=2):
    """Amortized per-execution device time (s) via pipelined dispatch slope.

    Requires a prior kernel() call (persistent executable + device inputs).
    """
    import time
    ex = _cache['exec']
    jax = ex['jax']
    dev_in = _cache['last_dev_in']

    def run_m(m):
        t0 = time.time()
        outs = [ex['run'](*dev_in, *ex['dev_zeros']) for _ in range(m)]
        jax.block_until_ready(outs)
        return time.time() - t0

    for _ in range(warmup):
        run_m(1)
    m1, m2 = 4, iters
    t1 = min(run_m(m1), run_m(m1))
    t2 = min(run_m(m2), run_m(m2))
    slope = (t2 - t1) / (m2 - m1)
    single = min(run_m(1), run_m(1))
    return slope, single


# revision 12
# speedup vs baseline: 2542.8774x; 1.2845x over previous
"""5-layer GAT (4x GATConv 128->128 heads=4, then GATConv 128->64 heads=1)
on 8 trn2 NeuronCores.

Sharding: edges partitioned by dst node across cores (each core owns 6272 dst
nodes = 49 blocks of 128). Per layer, a replicated node-feature table
[h | s_src] lives in shared DRAM, rebuilt each layer via AllGather of per-core
slices. Each core fetches h[src] rows for its edges with dma_gather (512B rows
layers 0-3, 256B bf16 rows layer 4; 4 SWDGE queues, int16 indices over a
lo/hi table split), computes edge softmax numerators, and aggregates into
per-block PSUM accumulators via one-hot matmuls (edges pre-sorted by dst on
the host, so each 128-edge tile belongs to one 128-node block).

The edge pipeline is fused: per batch of up to 8 tiles (1024 edges) it does
one tensor_tensor is_equal for all slot->node one-hots (S4), PE transposes +
one PSUM->SBUF copy for the node->slot one-hots (ST), per-tile sde matmuls,
one add, Prelu+Exp on the scalar engine, and a single broadcast multiply for
all messages.

Execution: a persistent jitted PJRT executable is cached along with
device-resident edge-derived constant inputs; per call only x and the small
weights are re-shipped.
"""
import sys
sys.path.insert(0, '/opt/trn_rl_repo')

import numpy as np

import concourse.bass as bass
import concourse.bacc as bacc
import concourse.tile as tile
import concourse.mybir as mybir
from concourse.masks import make_identity

N = 50000
E = 1_600_000
IN = 128
HID = 32
HEADS = 4
HC = HEADS * HID          # 128
OUT = 64
NEG = 0.2

CORES = 8
NPC = 6272                # nodes per core
NB = CORES * NPC          # 50176
NBLK = NPC // 128         # 49
HALF = NB // 2            # 25088 (int16-index table split, chunk-major rows)
NCHUNK = 7                # AllGather chunks per layer (49 = 7 x 7 blocks)
CBLK = NBLK // NCHUNK     # blocks per chunk
CROWS = CORES * CBLK * 128  # table rows per chunk (7168)
TCOLS = 256               # bf16 table row: [h(128) | s_src(4) | pad] = 512B
TCOLS4 = 128              # bf16 table row: [h5(64) | s_src5(1) | pad] = 256B
MAXB = 8                  # tiles per gather batch (1024 idxs)
NSWQ = 4

dt = mybir.dt
f32 = dt.float32
bf16 = dt.bfloat16

_cache = {}


def _rap(ap, free_dims):
    """Raw AP: keep partition dim of `ap`, replace free dims with [step,count] list."""
    return bass.AP(ap.tensor, ap.offset, [list(ap.ap[0])] + [list(d) for d in free_dims])


# ---------------------------------------------------------------- host prep

def _perm_rows(nodes):
    """Global node id -> chunk-major table row (AllGather chunk layout)."""
    k = nodes // NPC
    r = nodes % NPC
    b = r // 128
    lane = r % 128
    c = b // CBLK
    bl = b % CBLK
    return c * CROWS + k * (CBLK * 128) + bl * 128 + lane


def _prep(edge_index):
    src = np.asarray(edge_index[0], dtype=np.int64)
    dst = np.asarray(edge_index[1], dtype=np.int64)

    core = dst // NPC
    blk = (dst % NPC) // 128
    dl_val = (dst % NPC) % 128
    srcp = _perm_rows(src)
    half = (srcp >= HALF).astype(np.int64)

    key = (core * NBLK + blk) * 2 + half
    cnt = np.bincount(key, minlength=CORES * NBLK * 2).reshape(CORES, NBLK, 2)
    tiles_per = np.maximum(np.ceil(cnt / 128).astype(np.int64).max(axis=0), 1)  # [NBLK,2]
    T_LO, T_HI = tiles_per[:, 0], tiles_per[:, 1]
    TOT_TILES = int((T_LO + T_HI).sum())
    TOT_SLOTS = TOT_TILES * 128

    group_tiles = tiles_per.reshape(-1)                              # [NBLK*2]
    group_base = np.concatenate([[0], np.cumsum(group_tiles)[:-1]]) * 128

    # batch schedule: (block, half, nt, tile0, slot0); batches ordered by slot
    sched = []
    tcur = 0
    for b in range(NBLK):
        for h in range(2):
            ntiles = int(group_tiles[b * 2 + h])
            done = 0
            while done < ntiles:
                nt = min(MAXB, ntiles - done)
                sched.append((b, h, nt, tcur, int(group_base[b * 2 + h]) + done * 128))
                tcur += nt
                done += nt
    assert tcur == TOT_TILES
    NBATCH = len(sched)

    per_core = []
    for k in range(CORES):
        m = core == k
        s_k, blk_k, dl_k, half_k = srcp[m], blk[m], dl_val[m], half[m]
        gkey = blk_k * 2 + half_k
        order = np.argsort(gkey, kind='stable')
        s_k, dl_k, gkey = s_k[order], dl_k[order], gkey[order]
        gcnt = np.bincount(gkey, minlength=NBLK * 2)
        starts = np.concatenate([[0], np.cumsum(gcnt)[:-1]])
        rank = np.arange(len(gkey)) - starts[gkey]
        slot = group_base[gkey] + rank

        src_slot = np.zeros(TOT_SLOTS, dtype=np.int64)               # pad -> row 0
        dl_slot = np.full(TOT_SLOTS, -1.0, dtype=np.float32)         # pad -> -1
        src_slot[slot] = np.where(s_k >= HALF, s_k - HALF, s_k)
        dl_slot[slot] = dl_k.astype(np.float32)

        # wrapped int16 indices: per batch, idx i -> partition i%16, col i//16;
        # replicated into all 8 groups of 16 partitions
        seg_all = src_slot.astype(np.int16).reshape(TOT_SLOTS // 16, 16).T  # [16, S/16]
        idx16 = np.tile(seg_all, (8, 1))                             # [128, S/16]

        dl_arr = np.ascontiguousarray(dl_slot.reshape(TOT_TILES, 128).T)  # [128, T]
        per_core.append((idx16, dl_arr))

    return sched, T_LO, T_HI, TOT_TILES, NBATCH, per_core


def _prep_weights(W_stack, asrc_stack, adst_stack, b_stack,
                  W_last, asrc_last, adst_last, b_last):
    wcat = np.zeros((4, IN, 136), dtype=np.float32)
    for l in range(4):
        W = np.asarray(W_stack[l], dtype=np.float32)
        As = np.zeros((HC, HEADS), dtype=np.float32)
        Ad = np.zeros((HC, HEADS), dtype=np.float32)
        for h in range(HEADS):
            As[h * HID:(h + 1) * HID, h] = np.asarray(asrc_stack[l][h])
            Ad[h * HID:(h + 1) * HID, h] = np.asarray(adst_stack[l][h])
        wcat[l, :, :HC] = W
        wcat[l, :, HC:HC + HEADS] = W @ As
        wcat[l, :, HC + HEADS:] = W @ Ad
    WL = np.asarray(W_last, dtype=np.float32)
    wcat4 = np.zeros((HC, 66), dtype=np.float32)
    wcat4[:, :OUT] = WL
    wcat4[:, OUT] = WL @ np.asarray(asrc_last, dtype=np.float32)[0]
    wcat4[:, OUT + 1] = WL @ np.asarray(adst_last, dtype=np.float32)[0]
    bias = np.tile(np.asarray(b_stack, dtype=np.float32)[:, None, :], (1, 128, 1))
    bias4 = np.tile(np.asarray(b_last, dtype=np.float32)[None, :], (128, 1))
    return wcat, wcat4, bias, bias4


# ---------------------------------------------------------------- device program

def _build(sched, T_LO, T_HI, TOT_TILES, NBATCH):
    IDX_COLS = TOT_TILES * 8
    nc = bacc.Bacc("TRN2", target_bir_lowering=False, debug=False,
                   num_devices=CORES, num_swdge_queues=NSWQ)

    xs = nc.dram_tensor("xs", [NPC, IN], f32, kind="ExternalInput")
    idx16_in = nc.dram_tensor("idx16", [128, IDX_COLS], dt.int16, kind="ExternalInput")
    dl_in = nc.dram_tensor("dl", [128, TOT_TILES], f32, kind="ExternalInput")
    wcat_in = nc.dram_tensor("wcat", [4, IN, 136], f32, kind="ExternalInput")
    wcat4_in = nc.dram_tensor("wcat4", [HC, 66], f32, kind="ExternalInput")
    bias_in = nc.dram_tensor("bias", [4, 128, 128], f32, kind="ExternalInput")
    bias4_in = nc.dram_tensor("bias4", [128, OUT], f32, kind="ExternalInput")
    out_ext = nc.dram_tensor("out", [NPC, OUT], f32, kind="ExternalOutput")

    tbl = [nc.dram_tensor(f"tbl{l}", [NB, TCOLS], bf16, kind="Internal",
                          addr_space="Shared") for l in range(4)]
    tbl4 = nc.dram_tensor("tbl4", [NB, TCOLS4], bf16, kind="Internal",
                          addr_space="Shared")
    hb = [nc.dram_tensor(f"hb{l}", [NPC, TCOLS], bf16, kind="Internal")
          for l in range(4)]
    hb4 = nc.dram_tensor("hb4", [NPC, TCOLS4], bf16, kind="Internal")

    RG = [list(range(CORES))]

    with tile.TileContext(nc) as tc:
        with tc.tile_pool(name="const", bufs=1) as cpool, \
             tc.tile_pool(name="work", bufs=3) as wpool, \
             tc.tile_pool(name="gbuf", bufs=6) as gpool, \
             tc.tile_pool(name="spool", bufs=4) as spool, \
             tc.tile_pool(name="psA", bufs=2, space="PSUM") as psA, \
             tc.tile_pool(name="psB", bufs=2, space="PSUM") as psB, \
             tc.tile_pool(name="psC", bufs=1, space="PSUM") as psC:

            # ---- constants
            iota_row_i = cpool.tile([128, 128], dt.int32)
            nc.gpsimd.iota(iota_row_i[:], pattern=[[1, 128]], base=0, channel_multiplier=0)
            iota_row = cpool.tile([128, 128], bf16)
            nc.vector.tensor_copy(iota_row[:], iota_row_i[:])
            ident = cpool.tile([128, 128], f32)
            make_identity(nc, ident[:])
            ident_bf = cpool.tile([128, 128], bf16)
            nc.vector.tensor_copy(ident_bf[:], ident[:])

            idx_sb = cpool.tile([128, IDX_COLS], dt.int16)
            nc.sync.dma_start(idx_sb[:], idx16_in[:])
            dl_f = cpool.tile([128, TOT_TILES], f32)
            nc.sync.dma_start(dl_f[:], dl_in[:])
            dl_sb = cpool.tile([128, TOT_TILES], bf16)
            nc.vector.tensor_copy(dl_sb[:], dl_f[:])

            wcat_sb = cpool.tile([128, 4 * 136], bf16)
            for l in range(4):
                nc.gpsimd.dma_start(wcat_sb[:, l * 136:(l + 1) * 136], wcat_in[l])
            wcat4_sb = cpool.tile([128, 66], bf16)
            nc.gpsimd.dma_start(wcat4_sb[:], wcat4_in[:])
            bias_sb = cpool.tile([128, 4 * 128], f32)
            for l in range(4):
                nc.sync.dma_start(bias_sb[:, l * 128:(l + 1) * 128], bias_in[l])
            bias4_sb = cpool.tile([128, OUT], f32)
            nc.sync.dma_start(bias4_sb[:], bias4_in[:])

            sdst_sb = [cpool.tile([128, NBLK * 4], bf16, tag=f"sdst{i}",
                                  name=f"sdst{i}") for i in range(2)]
            sdst4_sb = cpool.tile([128, NBLK], bf16)

            def node_phase(l, b, act_ap):
                """Project block-b activations into layer-l table staging + s_dst."""
                tp = psC.tile([128, 128], f32, tag="tp")
                nc.tensor.transpose(tp[:], act_ap, ident[:])
                actT = wpool.tile([128, 128], bf16, tag="actT")
                nc.vector.tensor_copy(actT[:], tp[:])
                if l < 4:
                    ntp = psC.tile([128, 136], f32, tag="ntp")
                    nc.tensor.matmul(ntp[:], lhsT=actT[:],
                                     rhs=wcat_sb[:, l * 136:(l + 1) * 136],
                                     start=True, stop=True)
                    stage = wpool.tile([128, 132], bf16, tag="stage")
                    nc.vector.tensor_copy(stage[:], ntp[:, 0:132])
                    nc.scalar.copy(sdst_sb[l % 2][:, 4 * b:4 * b + 4], ntp[:, 132:136])
                    nc.sync.dma_start(hb[l][b * 128:(b + 1) * 128, 0:132], stage[:])
                else:
                    ntp = psC.tile([128, 66], f32, tag="ntp")
                    nc.tensor.matmul(ntp[:], lhsT=actT[:], rhs=wcat4_sb[:],
                                     start=True, stop=True)
                    stage4 = wpool.tile([128, 65], bf16, tag="stage4")
                    nc.vector.tensor_copy(stage4[:], ntp[:, 0:65])
                    nc.scalar.copy(sdst4_sb[:, b:b + 1], ntp[:, 65:66])
                    nc.sync.dma_start(hb4[b * 128:(b + 1) * 128, 0:65], stage4[:])

            def chunk_ag(lp, c):
                """AllGather chunk c (blocks c*CBLK..) of layer-lp staging."""
                src_hb = hb[lp] if lp < 4 else hb4
                dst_tbl = tbl[lp] if lp < 4 else tbl4
                rs, re = c * CBLK * 128, (c + 1) * CBLK * 128
                os_, oe = c * CROWS, (c + 1) * CROWS
                nc.gpsimd.collective_compute(
                    "AllGather", mybir.AluOpType.bypass, replica_groups=RG,
                    ins=[src_hb[rs:re, :].opt()],
                    outs=[dst_tbl[os_:oe, :].opt()])

            # ---- layer 0 node phase: build table0 from xs
            for b in range(NBLK):
                xt = wpool.tile([128, 128], f32, tag="xt")
                nc.sync.dma_start(xt[:], xs[b * 128:(b + 1) * 128, :])
                node_phase(0, b, xt[:])
                if (b + 1) % CBLK == 0:
                    chunk_ag(0, b // CBLK)

            # ---- per-block grouping of the batch schedule
            blocks = []
            for i, ent in enumerate(sched):
                if not blocks or ent[0] != blocks[-1][-1][1][0]:
                    blocks.append([])
                blocks[-1].append((i, ent))

            qrot = [0]

            def edge_layer(l):
                final = l == 4
                nh = 1 if final else HEADS
                ch = OUT if final else HID
                mc = nh * ch + nh                  # 65 or 132
                table = tbl4 if final else tbl[l]
                elem = TCOLS4 if final else TCOLS
                scol = nh * ch                     # s_src col in table row

                for batches in blocks:
                    b = batches[0][1][0]
                    ntiles_b = int(T_LO[b] + T_HI[b])
                    pblk = psA.tile([128, mc], f32, tag="pblk")
                    first = True
                    done_t = 0
                    for (bidx, (_b, hf, nt, t0, _slot0)) in batches:
                        G = nt * 128
                        gb = gpool.tile([128, MAXB * TCOLS], bf16, tag="gb")
                        tin = table[HALF:NB, :] if hf else table[0:HALF, :]
                        nc.gpsimd.dma_gather(
                            out_ap=_rap(gb[:], [[elem, nt], [1, elem]]),
                            in_ap=tin,
                            idxs_ap=idx_sb[:, t0 * 8:t0 * 8 + G // 16],
                            num_idxs=G, num_idxs_reg=G, elem_size=elem,
                            transpose=False, queue_num=qrot[0] % NSWQ)
                        qrot[0] += 1

                        # S4[p=slot, (j, node)] = (node == dl[slot of tile j])
                        S4 = spool.tile([128, MAXB * 128], bf16, tag="S4")
                        nc.vector.tensor_tensor(
                            out=_rap(S4[:], [[128, nt], [1, 128]]),
                            in0=_rap(iota_row[:], [[0, nt], [1, 128]]),
                            in1=_rap(dl_sb[:, t0:t0 + 1], [[1, nt], [0, 128]]),
                            op=mybir.AluOpType.is_equal)
                        # ST = per-tile transpose of S4 (node -> slot one-hot)
                        STp = psB.tile([128, MAXB * 128], bf16, tag="STp")
                        for j in range(nt):
                            nc.tensor.transpose(STp[:, j * 128:(j + 1) * 128],
                                                S4[:, j * 128:(j + 1) * 128],
                                                ident_bf[:])
                        ST = spool.tile([128, MAXB * 128], bf16, tag="ST")
                        nc.scalar.copy(ST[:, 0:nt * 128], STp[:, 0:nt * 128])
                        # sde[slot, (j,h)] = s_dst[dl[slot], h]
                        sde = psB.tile([128, MAXB * 4], f32, tag="sde")
                        for j in range(nt):
                            nc.tensor.matmul(
                                sde[:, j * nh:(j + 1) * nh],
                                lhsT=ST[:, j * 128:(j + 1) * 128],
                                rhs=(sdst4_sb[:, b:b + 1] if final
                                     else sdst_sb[l % 2][:, 4 * b:4 * b + 4]),
                                start=True, stop=True)

                        # scores: sc = sde + gathered s_src
                        sc = wpool.tile([128, MAXB * 4], f32, tag="sc")
                        nc.vector.tensor_tensor(
                            out=_rap(sc[:], [[nh, nt], [1, nh]]),
                            in0=_rap(sde[:], [[nh, nt], [1, nh]]),
                            in1=_rap(gb[:, scol:scol + 1], [[elem, nt], [1, nh]]),
                            op=mybir.AluOpType.add)
                        # leaky relu + exp on the scalar engine; exp lands in
                        # the msg numerator columns directly
                        lr = wpool.tile([128, MAXB * 4], f32, tag="lr")
                        nc.scalar.activation(lr[:, 0:nt * nh], sc[:, 0:nt * nh],
                                             mybir.ActivationFunctionType.Prelu,
                                             alpha=NEG)
                        msg = gpool.tile([128, MAXB * mc], bf16, tag="msg")
                        nc.scalar.activation(
                            _rap(msg[:, scol:scol + 1], [[mc, nt], [1, nh]]),
                            lr[:, 0:nt * nh],
                            mybir.ActivationFunctionType.Exp)
                        # msg[:, j, h*ch:(h+1)*ch] = gb * ex, one broadcast op
                        nc.vector.tensor_tensor(
                            out=_rap(msg[:], [[mc, nt], [ch, nh], [1, ch]]),
                            in0=_rap(gb[:], [[elem, nt], [ch, nh], [1, ch]]),
                            in1=_rap(msg[:, scol:scol + 1], [[mc, nt], [1, nh], [0, ch]]),
                            op=mybir.AluOpType.mult)

                        for j in range(nt):
                            nc.tensor.matmul(
                                pblk[:],
                                lhsT=S4[:, j * 128:(j + 1) * 128],
                                rhs=_rap(msg[:, j * mc:j * mc + 1], [[1, mc]]),
                                start=first,
                                stop=(done_t + j == ntiles_b - 1))
                            first = False
                        done_t += nt

                    # ---- block epilogue
                    rec = wpool.tile([128, 4], f32, tag="rec")
                    nc.vector.tensor_scalar(out=rec[:, 0:nh], in0=pblk[:, nh * ch:nh * ch + nh],
                                            scalar1=1e-16, scalar2=None,
                                            op0=mybir.AluOpType.add)
                    nc.vector.reciprocal(rec[:, 0:nh], rec[:, 0:nh])
                    act = wpool.tile([128, 128], f32, tag="act")
                    nc.vector.tensor_tensor(
                        out=_rap(act[:], [[ch, nh], [1, ch]]),
                        in0=_rap(pblk[:], [[ch, nh], [1, ch]]),
                        in1=_rap(rec[:], [[1, nh], [0, ch]]),
                        op=mybir.AluOpType.mult)
                    if final:
                        nc.vector.tensor_tensor(out=act[:, 0:OUT], in0=act[:, 0:OUT],
                                                in1=bias4_sb[:], op=mybir.AluOpType.add)
                        nc.sync.dma_start(out_ext[b * 128:(b + 1) * 128, :], act[:, 0:OUT])
                    else:
                        nc.vector.tensor_tensor(out=act[:], in0=act[:],
                                                in1=bias_sb[:, l * 128:(l + 1) * 128],
                                                op=mybir.AluOpType.add)
                        neg = wpool.tile([128, 128], f32, tag="neg")
                        nc.vector.tensor_scalar(out=neg[:], in0=act[:], scalar1=0.0,
                                                scalar2=None, op0=mybir.AluOpType.min)
                        en = wpool.tile([128, 128], f32, tag="en")
                        nc.scalar.activation(en[:], neg[:], mybir.ActivationFunctionType.Exp)
                        pos = wpool.tile([128, 128], f32, tag="pos")
                        nc.scalar.activation(pos[:], act[:], mybir.ActivationFunctionType.Relu)
                        nc.vector.tensor_tensor(out=act[:], in0=en[:], in1=pos[:],
                                                op=mybir.AluOpType.add)
                        nc.vector.tensor_scalar(out=act[:], in0=act[:], scalar1=-1.0,
                                                scalar2=None, op0=mybir.AluOpType.add)
                        node_phase(l + 1, b, act[:])
                        if (b + 1) % CBLK == 0:
                            chunk_ag(l + 1, b // CBLK)

            for l in range(5):
                edge_layer(l)

    nc.compile()
    return nc


# ---------------------------------------------------------------- executor

def _make_exec(nc):
    """Build a persistent jitted PJRT executable for nc (multi-core SPMD)."""
    import jax
    from jax.sharding import Mesh, PartitionSpec, NamedSharding
    import warnings
    with warnings.catch_warnings():
        warnings.simplefilter("ignore")
        from jax.experimental.shard_map import shard_map
    from concourse.bass2jax import (_bass_exec_p, install_neuronx_cc_hook,
                                    partition_id_tensor)

    install_neuronx_cc_hook()
    partition_name = nc.partition_id_tensor.name if nc.partition_id_tensor else None
    in_names, out_names, out_avals, zero_outs = [], [], [], []
    for alloc in nc.m.functions[0].allocations:
        if not isinstance(alloc, mybir.MemoryLocationSet):
            continue
        name = alloc.memorylocations[0].name
        if alloc.kind == "ExternalInput":
            if name != partition_name:
                in_names.append(name)
        elif alloc.kind == "ExternalOutput":
            out_names.append(name)
            shape = tuple(alloc.tensor_shape)
            dtype = mybir.dt.np(alloc.dtype)
            out_avals.append(jax.core.ShapedArray(shape, dtype))
            zero_outs.append(np.zeros(shape, dtype))
    n_params = len(in_names)
    n_outs = len(out_avals)
    in_names_all = in_names + out_names + ([partition_name] if partition_name else [])

    def _body(*args):
        operands = list(args)
        if partition_name is not None:
            operands.append(partition_id_tensor())
        outs = _bass_exec_p.bind(
            *operands, out_avals=tuple(out_avals),
            in_names=tuple(in_names_all), out_names=tuple(out_names),
            lowering_input_output_aliases=(), sim_require_finite=True,
            sim_require_nnan=True, nc=nc)
        return tuple(outs)

    devices = jax.devices()[:CORES]
    mesh = Mesh(np.asarray(devices), ("core",))
    sh = NamedSharding(mesh, PartitionSpec("core"))
    in_specs = (PartitionSpec("core"),) * (n_params + n_outs)
    out_specs = (PartitionSpec("core"),) * n_outs
    run = jax.jit(
        shard_map(_body, mesh=mesh, in_specs=in_specs, out_specs=out_specs,
                  check_rep=False),
        keep_unused=True)
    dev_zeros = jax.device_put(
        [np.zeros((CORES * z.shape[0], *z.shape[1:]), z.dtype) for z in zero_outs],
        [sh] * n_outs)
    return dict(run=run, sharding=sh, in_names=in_names, out_names=out_names,
                dev_zeros=dev_zeros, jax=jax)


# ---------------------------------------------------------------- entry point

def kernel(x, edge_index, W_stack, asrc_stack, adst_stack, b_stack,
           W_last, asrc_last, adst_last, b_last):
    ek = np.asarray(edge_index)
    ckb = ek.tobytes()
    ck = (len(ckb), ckb[:512], ckb[-512:])
    if 'exec' not in _cache or _cache.get('ck') != ck:
        sched, T_LO, T_HI, TOT_TILES, NBATCH, per_core = _prep(ek)
        nc = _build(sched, T_LO, T_HI, TOT_TILES, NBATCH)
        ex = _make_exec(nc)
        jax = ex['jax']
        # edge-derived constants live on device across calls
        const_dev = {
            "idx16": jax.device_put(
                np.concatenate([per_core[k][0] for k in range(CORES)], axis=0),
                ex['sharding']),
            "dl": jax.device_put(
                np.concatenate([per_core[k][1] for k in range(CORES)], axis=0),
                ex['sharding']),
        }
        jax.block_until_ready(list(const_dev.values()))
        ex['const_dev'] = const_dev
        _cache.update(**{'exec': ex, 'ck': ck})
    ex = _cache['exec']
    jax = ex['jax']

    wcat, wcat4, bias, bias4 = _prep_weights(
        W_stack, asrc_stack, adst_stack, b_stack,
        W_last, asrc_last, adst_last, b_last)

    x_np = np.asarray(x, dtype=np.float32)
    xs_pad = np.zeros((NB, IN), dtype=np.float32)
    xs_pad[:N] = x_np

    host_in = {
        "xs": xs_pad,
        "wcat": np.tile(wcat, (CORES, 1, 1)),
        "wcat4": np.tile(wcat4, (CORES, 1)),
        "bias": np.tile(bias, (CORES, 1, 1)),
        "bias4": np.tile(bias4, (CORES, 1)),
    }
    dev_in = []
    for name in ex['in_names']:
        if name in ex['const_dev']:
            dev_in.append(ex['const_dev'][name])
        else:
            dev_in.append(jax.device_put(host_in[name], ex['sharding']))
    outs = ex['run'](*dev_in, *ex['dev_zeros'])
    _cache['last_dev_in'] = dev_in
    out_idx = ex['out_names'].index("out")
    out = np.asarray(outs[out_idx]).reshape(CORES, NPC, OUT).reshape(CORES * NPC, OUT)
    return out[:N].astype(np.float32)


def bench_exec(iters=24, warmup=2):
    """Amortized per-execution device time (s) via pipelined dispatch slope.

    Requires a prior kernel() call (persistent executable + device inputs).
    """
    import time
    ex = _cache['exec']
    jax = ex['jax']
    dev_in = _cache['last_dev_in']

    def run_m(m):
        t0 = time.time()
        outs = [ex['run'](*dev_in, *ex['dev_zeros']) for _ in range(m)]
        jax.block_until_ready(outs)
        return time.time() - t0

    for _ in range(warmup):
        run_m(1)
    m1, m2 = 4, iters
    t1 = min(run_m(m1), run_m(m1))
    t2 = min(run_m(m2), run_m(m2))
    slope = (t2 - t1) / (m2 - m1)
    single = min(run_m(1), run_m(1))
    return slope, single


# revision 17
# speedup vs baseline: 2710.4521x; 1.0659x over previous
"""5-layer GAT (4x GATConv 128->128 heads=4, then GATConv 128->64 heads=1)
on 8 trn2 NeuronCores.

Sharding: edges partitioned by dst node across cores (each core owns 6272 dst
nodes = 49 blocks of 128). Per layer, a replicated node-feature table
[h | s_src] lives in shared DRAM, rebuilt each layer via AllGather of per-core
slices. Each core fetches h[src] rows for its edges with dma_gather (512B rows
layers 0-3, 256B bf16 rows layer 4; 4 SWDGE queues, int16 indices over a
lo/hi table split), computes edge softmax numerators, and aggregates into
per-block PSUM accumulators via one-hot matmuls (edges pre-sorted by dst on
the host, so each 128-edge tile belongs to one 128-node block).

The edge pipeline is fused: per batch of up to 8 tiles (1024 edges) it does
one tensor_tensor is_equal for all slot->node one-hots (S4), PE transposes +
one PSUM->SBUF copy for the node->slot one-hots (ST), per-tile sde matmuls,
one add, Prelu+Exp on the scalar engine, and a single broadcast multiply for
all messages.

Execution: a persistent jitted PJRT executable is cached along with
device-resident edge-derived constant inputs; per call only x and the small
weights are re-shipped.
"""
import sys
sys.path.insert(0, '/opt/trn_rl_repo')

import numpy as np

import concourse.bass as bass
import concourse.bacc as bacc
import concourse.tile as tile
import concourse.mybir as mybir
from concourse.masks import make_identity

N = 50000
E = 1_600_000
IN = 128
HID = 32
HEADS = 4
HC = HEADS * HID          # 128
OUT = 64
NEG = 0.2

CORES = 8
NPC = 6272                # nodes per core
NB = CORES * NPC          # 50176
NBLK = NPC // 128         # 49
HALF = NB // 2            # 25088 (int16-index table split, chunk-major rows)
NCHUNK = 7                # AllGather chunks per layer (49 = 7 x 7 blocks)
CBLK = NBLK // NCHUNK     # blocks per chunk
CROWS = CORES * CBLK * 128  # table rows per chunk (7168)
TCOLS = 256               # bf16 table row: [h(128) | s_src(4) | pad] = 512B
TCOLS4 = 128              # bf16 table row: [h5(64) | s_src5(1) | pad] = 256B
MAXB = 8                  # tiles per gather batch (1024 idxs)
NSWQ = 4

dt = mybir.dt
f32 = dt.float32
bf16 = dt.bfloat16

_cache = {}


def _rap(ap, free_dims):
    """Raw AP: keep partition dim of `ap`, replace free dims with [step,count] list."""
    return bass.AP(ap.tensor, ap.offset, [list(ap.ap[0])] + [list(d) for d in free_dims])


# ---------------------------------------------------------------- host prep

def _perm_rows(nodes):
    """Global node id -> chunk-major table row (AllGather chunk layout)."""
    k = nodes // NPC
    r = nodes % NPC
    b = r // 128
    lane = r % 128
    c = b // CBLK
    bl = b % CBLK
    return c * CROWS + k * (CBLK * 128) + bl * 128 + lane


def _prep(edge_index):
    src = np.asarray(edge_index[0], dtype=np.int64)
    dst = np.asarray(edge_index[1], dtype=np.int64)

    core = dst // NPC
    blk = (dst % NPC) // 128
    dl_val = (dst % NPC) % 128
    srcp = _perm_rows(src)
    half = (srcp >= HALF).astype(np.int64)

    key = (core * NBLK + blk) * 2 + half
    cnt = np.bincount(key, minlength=CORES * NBLK * 2).reshape(CORES, NBLK, 2)
    tiles_per = np.maximum(np.ceil(cnt / 128).astype(np.int64).max(axis=0), 1)  # [NBLK,2]
    T_LO, T_HI = tiles_per[:, 0], tiles_per[:, 1]
    TOT_TILES = int((T_LO + T_HI).sum())
    TOT_SLOTS = TOT_TILES * 128

    group_tiles = tiles_per.reshape(-1)                              # [NBLK*2]
    group_base = np.concatenate([[0], np.cumsum(group_tiles)[:-1]]) * 128

    # batch schedule: (block, nt, tile0, n_lo); batches may straddle the
    # lo/hi boundary of a block (two gathers, one fused compute pass)
    sched = []
    tcur = 0
    for b in range(NBLK):
        ntiles = int(T_LO[b] + T_HI[b])
        t_base = tcur
        done = 0
        while done < ntiles:
            nt = min(MAXB, ntiles - done)
            t0 = t_base + done
            nlo = int(min(max(T_LO[b] - done, 0), nt))
            sched.append((b, nt, t0, nlo))
            done += nt
        tcur += ntiles
    assert tcur == TOT_TILES
    NBATCH = len(sched)

    per_core = []
    for k in range(CORES):
        m = core == k
        s_k, blk_k, dl_k, half_k = srcp[m], blk[m], dl_val[m], half[m]
        gkey = blk_k * 2 + half_k
        order = np.argsort(gkey, kind='stable')
        s_k, dl_k, gkey = s_k[order], dl_k[order], gkey[order]
        gcnt = np.bincount(gkey, minlength=NBLK * 2)
        starts = np.concatenate([[0], np.cumsum(gcnt)[:-1]])
        rank = np.arange(len(gkey)) - starts[gkey]
        slot = group_base[gkey] + rank

        src_slot = np.zeros(TOT_SLOTS, dtype=np.int64)               # pad -> row 0
        dl_slot = np.full(TOT_SLOTS, -1.0, dtype=np.float32)         # pad -> -1
        src_slot[slot] = np.where(s_k >= HALF, s_k - HALF, s_k)
        dl_slot[slot] = dl_k.astype(np.float32)

        # wrapped int16 indices: per batch, idx i -> partition i%16, col i//16;
        # replicated into all 8 groups of 16 partitions
        seg_all = src_slot.astype(np.int16).reshape(TOT_SLOTS // 16, 16).T  # [16, S/16]
        idx16 = np.tile(seg_all, (8, 1))                             # [128, S/16]

        dl_arr = np.ascontiguousarray(dl_slot.reshape(TOT_TILES, 128).T)  # [128, T]
        per_core.append((idx16, dl_arr))

    return sched, T_LO, T_HI, TOT_TILES, NBATCH, per_core


def _prep_weights(W_stack, asrc_stack, adst_stack, b_stack,
                  W_last, asrc_last, adst_last, b_last):
    wcat = np.zeros((4, IN, 136), dtype=np.float32)
    for l in range(4):
        W = np.asarray(W_stack[l], dtype=np.float32)
        As = np.zeros((HC, HEADS), dtype=np.float32)
        Ad = np.zeros((HC, HEADS), dtype=np.float32)
        for h in range(HEADS):
            As[h * HID:(h + 1) * HID, h] = np.asarray(asrc_stack[l][h])
            Ad[h * HID:(h + 1) * HID, h] = np.asarray(adst_stack[l][h])
        wcat[l, :, :HC] = W
        wcat[l, :, HC:HC + HEADS] = W @ As
        wcat[l, :, HC + HEADS:] = W @ Ad
    WL = np.asarray(W_last, dtype=np.float32)
    wcat4 = np.zeros((HC, 66), dtype=np.float32)
    wcat4[:, :OUT] = WL
    wcat4[:, OUT] = WL @ np.asarray(asrc_last, dtype=np.float32)[0]
    wcat4[:, OUT + 1] = WL @ np.asarray(adst_last, dtype=np.float32)[0]
    bias = np.tile(np.asarray(b_stack, dtype=np.float32)[:, None, :], (1, 128, 1))
    bias4 = np.tile(np.asarray(b_last, dtype=np.float32)[None, :], (128, 1))
    return wcat, wcat4, bias, bias4


# ---------------------------------------------------------------- device program

def _build(sched, T_LO, T_HI, TOT_TILES, NBATCH):
    IDX_COLS = TOT_TILES * 8
    nc = bacc.Bacc("TRN2", target_bir_lowering=False, debug=False,
                   num_devices=CORES, num_swdge_queues=NSWQ)

    xs = nc.dram_tensor("xs", [NPC, IN], f32, kind="ExternalInput")
    idx16_in = nc.dram_tensor("idx16", [128, IDX_COLS], dt.int16, kind="ExternalInput")
    dl_in = nc.dram_tensor("dl", [128, TOT_TILES], f32, kind="ExternalInput")
    wcat_in = nc.dram_tensor("wcat", [4, IN, 136], f32, kind="ExternalInput")
    wcat4_in = nc.dram_tensor("wcat4", [HC, 66], f32, kind="ExternalInput")
    bias_in = nc.dram_tensor("bias", [4, 128, 128], f32, kind="ExternalInput")
    bias4_in = nc.dram_tensor("bias4", [128, OUT], f32, kind="ExternalInput")
    out_ext = nc.dram_tensor("out", [NPC, OUT], f32, kind="ExternalOutput")

    tbl = [nc.dram_tensor(f"tbl{l}", [NB, TCOLS], bf16, kind="Internal",
                          addr_space="Shared") for l in range(4)]
    tbl4 = nc.dram_tensor("tbl4", [NB, TCOLS4], bf16, kind="Internal",
                          addr_space="Shared")
    hb = [nc.dram_tensor(f"hb{l}", [NPC, TCOLS], bf16, kind="Internal")
          for l in range(4)]
    hb4 = nc.dram_tensor("hb4", [NPC, TCOLS4], bf16, kind="Internal")

    RG = [list(range(CORES))]

    with tile.TileContext(nc) as tc:
        with tc.tile_pool(name="const", bufs=1) as cpool, \
             tc.tile_pool(name="work", bufs=4) as wpool, \
             tc.tile_pool(name="gbuf", bufs=6) as gpool, \
             tc.tile_pool(name="spool", bufs=6) as spool, \
             tc.tile_pool(name="psA", bufs=2, space="PSUM") as psA, \
             tc.tile_pool(name="psB", bufs=2, space="PSUM") as psB, \
             tc.tile_pool(name="psC", bufs=1, space="PSUM") as psC:

            # ---- constants
            iota_row_i = cpool.tile([128, 128], dt.int32)
            nc.gpsimd.iota(iota_row_i[:], pattern=[[1, 128]], base=0, channel_multiplier=0)
            iota_row = cpool.tile([128, 128], bf16)
            nc.vector.tensor_copy(iota_row[:], iota_row_i[:])
            ident = cpool.tile([128, 128], f32)
            make_identity(nc, ident[:])
            ident_bf = cpool.tile([128, 128], bf16)
            nc.vector.tensor_copy(ident_bf[:], ident[:])

            idx_sb = cpool.tile([128, IDX_COLS], dt.int16)
            nc.sync.dma_start(idx_sb[:], idx16_in[:])
            dl_f = cpool.tile([128, TOT_TILES], f32)
            nc.sync.dma_start(dl_f[:], dl_in[:])
            dl_sb = cpool.tile([128, TOT_TILES], bf16)
            nc.vector.tensor_copy(dl_sb[:], dl_f[:])

            wcat_sb = cpool.tile([128, 4 * 136], bf16)
            for l in range(4):
                nc.gpsimd.dma_start(wcat_sb[:, l * 136:(l + 1) * 136], wcat_in[l])
            wcat4_sb = cpool.tile([128, 66], bf16)
            nc.gpsimd.dma_start(wcat4_sb[:], wcat4_in[:])
            bias_sb = cpool.tile([128, 4 * 128], f32)
            for l in range(4):
                nc.sync.dma_start(bias_sb[:, l * 128:(l + 1) * 128], bias_in[l])
            bias4_sb = cpool.tile([128, OUT], f32)
            nc.sync.dma_start(bias4_sb[:], bias4_in[:])

            sdst_sb = [cpool.tile([128, NBLK * 4], bf16, tag=f"sdst{i}",
                                  name=f"sdst{i}") for i in range(2)]
            sdst4_sb = cpool.tile([128, NBLK], bf16)

            def node_phase(l, b, act_ap):
                """Project block-b activations into layer-l table staging + s_dst."""
                tp = psC.tile([128, 128], f32, tag="tp")
                nc.tensor.transpose(tp[:], act_ap, ident[:])
                actT = wpool.tile([128, 128], bf16, tag="actT")
                nc.vector.tensor_copy(actT[:], tp[:])
                if l < 4:
                    ntp = psC.tile([128, 136], f32, tag="ntp")
                    nc.tensor.matmul(ntp[:], lhsT=actT[:],
                                     rhs=wcat_sb[:, l * 136:(l + 1) * 136],
                                     start=True, stop=True)
                    stage = wpool.tile([128, 132], bf16, tag="stage")
                    nc.vector.tensor_copy(stage[:], ntp[:, 0:132])
                    nc.scalar.copy(sdst_sb[l % 2][:, 4 * b:4 * b + 4], ntp[:, 132:136])
                    nc.sync.dma_start(hb[l][b * 128:(b + 1) * 128, 0:132], stage[:])
                else:
                    ntp = psC.tile([128, 66], f32, tag="ntp")
                    nc.tensor.matmul(ntp[:], lhsT=actT[:], rhs=wcat4_sb[:],
                                     start=True, stop=True)
                    stage4 = wpool.tile([128, 65], bf16, tag="stage4")
                    nc.vector.tensor_copy(stage4[:], ntp[:, 0:65])
                    nc.scalar.copy(sdst4_sb[:, b:b + 1], ntp[:, 65:66])
                    nc.sync.dma_start(hb4[b * 128:(b + 1) * 128, 0:65], stage4[:])

            def chunk_ag(lp, c):
                """AllGather chunk c (blocks c*CBLK..) of layer-lp staging."""
                src_hb = hb[lp] if lp < 4 else hb4
                dst_tbl = tbl[lp] if lp < 4 else tbl4
                rs, re = c * CBLK * 128, (c + 1) * CBLK * 128
                os_, oe = c * CROWS, (c + 1) * CROWS
                nc.gpsimd.collective_compute(
                    "AllGather", mybir.AluOpType.bypass, replica_groups=RG,
                    ins=[src_hb[rs:re, :].opt()],
                    outs=[dst_tbl[os_:oe, :].opt()])

            # ---- layer 0 node phase: build table0 from xs
            for b in range(NBLK):
                xt = wpool.tile([128, 128], f32, tag="xt")
                nc.sync.dma_start(xt[:], xs[b * 128:(b + 1) * 128, :])
                node_phase(0, b, xt[:])
                if (b + 1) % CBLK == 0:
                    chunk_ag(0, b // CBLK)

            # ---- per-block grouping of the batch schedule
            blocks = []
            for i, ent in enumerate(sched):
                if not blocks or ent[0] != blocks[-1][-1][1][0]:
                    blocks.append([])
                blocks[-1].append((i, ent))

            qrot = [0, 0]   # separate rotation for lo (queues 0-1) / hi (2-3)

            def edge_layer(l):
                final = l == 4
                nh = 1 if final else HEADS
                ch = OUT if final else HID
                mc = nh * ch + nh                  # 65 or 132
                table = tbl4 if final else tbl[l]
                elem = TCOLS4 if final else TCOLS
                scol = nh * ch                     # s_src col in table row

                for batches in blocks:
                    b = batches[0][1][0]
                    ntiles_b = int(T_LO[b] + T_HI[b])
                    pblk = psA.tile([128, mc], f32, tag="pblk")
                    first = True
                    done_t = 0
                    for (bidx, (_b, nt, t0, nlo)) in batches:
                        gb = gpool.tile([128, MAXB * TCOLS], bf16, tag="gb")
                        if nlo:
                            G = nlo * 128
                            nc.gpsimd.dma_gather(
                                out_ap=_rap(gb[:], [[elem, nlo], [1, elem]]),
                                in_ap=table[0:HALF, :],
                                idxs_ap=idx_sb[:, t0 * 8:t0 * 8 + G // 16],
                                num_idxs=G, num_idxs_reg=G, elem_size=elem,
                                transpose=False, queue_num=qrot[0] % 2)
                            qrot[0] += 1
                        if nt > nlo:
                            G = (nt - nlo) * 128
                            th = t0 + nlo
                            nc.gpsimd.dma_gather(
                                out_ap=_rap(gb[:, nlo * elem:nlo * elem + 1],
                                            [[elem, nt - nlo], [1, elem]]),
                                in_ap=table[HALF:NB, :],
                                idxs_ap=idx_sb[:, th * 8:th * 8 + G // 16],
                                num_idxs=G, num_idxs_reg=G, elem_size=elem,
                                transpose=False, queue_num=2 + qrot[1] % 2)
                            qrot[1] += 1

                        # S4[p=slot, (j, node)] = (node == dl[slot of tile j])
                        S4 = spool.tile([128, MAXB * 128], bf16, tag="S4")
                        nc.vector.tensor_tensor(
                            out=_rap(S4[:], [[128, nt], [1, 128]]),
                            in0=_rap(iota_row[:], [[0, nt], [1, 128]]),
                            in1=_rap(dl_sb[:, t0:t0 + 1], [[1, nt], [0, 128]]),
                            op=mybir.AluOpType.is_equal)
                        # ST = per-tile transpose of S4 (node -> slot one-hot)
                        STp = psB.tile([128, MAXB * 128], bf16, tag="STp")
                        for j in range(nt):
                            nc.tensor.transpose(STp[:, j * 128:(j + 1) * 128],
                                                S4[:, j * 128:(j + 1) * 128],
                                                ident_bf[:])
                        ST = spool.tile([128, MAXB * 128], bf16, tag="ST")
                        nc.scalar.copy(ST[:, 0:nt * 128], STp[:, 0:nt * 128])
                        # sde[slot, (j,h)] = s_dst[dl[slot], h]
                        sde = psB.tile([128, MAXB * 4], f32, tag="sde")
                        for j in range(nt):
                            nc.tensor.matmul(
                                sde[:, j * nh:(j + 1) * nh],
                                lhsT=ST[:, j * 128:(j + 1) * 128],
                                rhs=(sdst4_sb[:, b:b + 1] if final
                                     else sdst_sb[l % 2][:, 4 * b:4 * b + 4]),
                                start=True, stop=True)

                        # scores: sc = sde + gathered s_src
                        sc = wpool.tile([128, MAXB * 4], f32, tag="sc")
                        nc.vector.tensor_tensor(
                            out=_rap(sc[:], [[nh, nt], [1, nh]]),
                            in0=_rap(sde[:], [[nh, nt], [1, nh]]),
                            in1=_rap(gb[:, scol:scol + 1], [[elem, nt], [1, nh]]),
                            op=mybir.AluOpType.add)
                        # leaky relu + exp on the scalar engine; exp lands in
                        # the msg numerator columns directly
                        lr = wpool.tile([128, MAXB * 4], f32, tag="lr")
                        nc.scalar.activation(lr[:, 0:nt * nh], sc[:, 0:nt * nh],
                                             mybir.ActivationFunctionType.Prelu,
                                             alpha=NEG)
                        msg = gpool.tile([128, MAXB * mc], bf16, tag="msg")
                        nc.scalar.activation(
                            _rap(msg[:, scol:scol + 1], [[mc, nt], [1, nh]]),
                            lr[:, 0:nt * nh],
                            mybir.ActivationFunctionType.Exp)
                        # msg[:, j, h*ch:(h+1)*ch] = gb * ex, one broadcast op
                        nc.vector.tensor_tensor(
                            out=_rap(msg[:], [[mc, nt], [ch, nh], [1, ch]]),
                            in0=_rap(gb[:], [[elem, nt], [ch, nh], [1, ch]]),
                            in1=_rap(msg[:, scol:scol + 1], [[mc, nt], [1, nh], [0, ch]]),
                            op=mybir.AluOpType.mult)

                        for j in range(nt):
                            nc.tensor.matmul(
                                pblk[:],
                                lhsT=S4[:, j * 128:(j + 1) * 128],
                                rhs=_rap(msg[:, j * mc:j * mc + 1], [[1, mc]]),
                                start=first,
                                stop=(done_t + j == ntiles_b - 1))
                            first = False
                        done_t += nt

                    # ---- block epilogue
                    rec = wpool.tile([128, 4], f32, tag="rec")
                    nc.vector.tensor_scalar(out=rec[:, 0:nh], in0=pblk[:, nh * ch:nh * ch + nh],
                                            scalar1=1e-16, scalar2=None,
                                            op0=mybir.AluOpType.add)
                    nc.vector.reciprocal(rec[:, 0:nh], rec[:, 0:nh])
                    act = wpool.tile([128, 128], f32, tag="act")
                    nc.vector.tensor_tensor(
                        out=_rap(act[:], [[ch, nh], [1, ch]]),
                        in0=_rap(pblk[:], [[ch, nh], [1, ch]]),
                        in1=_rap(rec[:], [[1, nh], [0, ch]]),
                        op=mybir.AluOpType.mult)
                    if final:
                        nc.vector.tensor_tensor(out=act[:, 0:OUT], in0=act[:, 0:OUT],
                                                in1=bias4_sb[:], op=mybir.AluOpType.add)
                        nc.sync.dma_start(out_ext[b * 128:(b + 1) * 128, :], act[:, 0:OUT])
                    else:
                        nc.vector.tensor_tensor(out=act[:], in0=act[:],
                                                in1=bias_sb[:, l * 128:(l + 1) * 128],
                                                op=mybir.AluOpType.add)
                        neg = wpool.tile([128, 128], f32, tag="neg")
                        nc.vector.tensor_scalar(out=neg[:], in0=act[:], scalar1=0.0,
                                                scalar2=None, op0=mybir.AluOpType.min)
                        en = wpool.tile([128, 128], f32, tag="en")
                        nc.scalar.activation(en[:], neg[:], mybir.ActivationFunctionType.Exp)
                        pos = wpool.tile([128, 128], f32, tag="pos")
                        nc.scalar.activation(pos[:], act[:], mybir.ActivationFunctionType.Relu)
                        nc.vector.tensor_tensor(out=act[:], in0=en[:], in1=pos[:],
                                                op=mybir.AluOpType.add)
                        nc.vector.tensor_scalar(out=act[:], in0=act[:], scalar1=-1.0,
                                                scalar2=None, op0=mybir.AluOpType.add)
                        node_phase(l + 1, b, act[:])
                        if (b + 1) % CBLK == 0:
                            chunk_ag(l + 1, b // CBLK)

            for l in range(5):
                edge_layer(l)

    nc.compile()
    return nc


# ---------------------------------------------------------------- executor

def _make_exec(nc):
    """Build a persistent jitted PJRT executable for nc (multi-core SPMD)."""
    import jax
    from jax.sharding import Mesh, PartitionSpec, NamedSharding
    import warnings
    with warnings.catch_warnings():
        warnings.simplefilter("ignore")
        from jax.experimental.shard_map import shard_map
    from concourse.bass2jax import (_bass_exec_p, install_neuronx_cc_hook,
                                    partition_id_tensor)

    install_neuronx_cc_hook()
    partition_name = nc.partition_id_tensor.name if nc.partition_id_tensor else None
    in_names, out_names, out_avals, zero_outs = [], [], [], []
    for alloc in nc.m.functions[0].allocations:
        if not isinstance(alloc, mybir.MemoryLocationSet):
            continue
        name = alloc.memorylocations[0].name
        if alloc.kind == "ExternalInput":
            if name != partition_name:
                in_names.append(name)
        elif alloc.kind == "ExternalOutput":
            out_names.append(name)
            shape = tuple(alloc.tensor_shape)
            dtype = mybir.dt.np(alloc.dtype)
            out_avals.append(jax.core.ShapedArray(shape, dtype))
            zero_outs.append(np.zeros(shape, dtype))
    n_params = len(in_names)
    n_outs = len(out_avals)
    in_names_all = in_names + out_names + ([partition_name] if partition_name else [])

    def _body(*args):
        operands = list(args)
        if partition_name is not None:
            operands.append(partition_id_tensor())
        outs = _bass_exec_p.bind(
            *operands, out_avals=tuple(out_avals),
            in_names=tuple(in_names_all), out_names=tuple(out_names),
            lowering_input_output_aliases=(), sim_require_finite=True,
            sim_require_nnan=True, nc=nc)
        return tuple(outs)

    devices = jax.devices()[:CORES]
    mesh = Mesh(np.asarray(devices), ("core",))
    sh = NamedSharding(mesh, PartitionSpec("core"))
    in_specs = (PartitionSpec("core"),) * (n_params + n_outs)
    out_specs = (PartitionSpec("core"),) * n_outs
    run = jax.jit(
        shard_map(_body, mesh=mesh, in_specs=in_specs, out_specs=out_specs,
                  check_rep=False),
        keep_unused=True)
    dev_zeros = jax.device_put(
        [np.zeros((CORES * z.shape[0], *z.shape[1:]), z.dtype) for z in zero_outs],
        [sh] * n_outs)
    return dict(run=run, sharding=sh, in_names=in_names, out_names=out_names,
                dev_zeros=dev_zeros, jax=jax)


# ---------------------------------------------------------------- entry point

def kernel(x, edge_index, W_stack, asrc_stack, adst_stack, b_stack,
           W_last, asrc_last, adst_last, b_last):
    ek = np.asarray(edge_index)
    ckb = ek.tobytes()
    ck = (len(ckb), ckb[:512], ckb[-512:])
    if 'exec' not in _cache or _cache.get('ck') != ck:
        sched, T_LO, T_HI, TOT_TILES, NBATCH, per_core = _prep(ek)
        nc = _build(sched, T_LO, T_HI, TOT_TILES, NBATCH)
        ex = _make_exec(nc)
        jax = ex['jax']
        # edge-derived constants live on device across calls
        const_dev = {
            "idx16": jax.device_put(
                np.concatenate([per_core[k][0] for k in range(CORES)], axis=0),
                ex['sharding']),
            "dl": jax.device_put(
                np.concatenate([per_core[k][1] for k in range(CORES)], axis=0),
                ex['sharding']),
        }
        jax.block_until_ready(list(const_dev.values()))
        ex['const_dev'] = const_dev
        _cache.update(**{'exec': ex, 'ck': ck})
    ex = _cache['exec']
    jax = ex['jax']

    wcat, wcat4, bias, bias4 = _prep_weights(
        W_stack, asrc_stack, adst_stack, b_stack,
        W_last, asrc_last, adst_last, b_last)

    x_np = np.asarray(x, dtype=np.float32)
    xs_pad = np.zeros((NB, IN), dtype=np.float32)
    xs_pad[:N] = x_np

    host_in = {
        "xs": xs_pad,
        "wcat": np.tile(wcat, (CORES, 1, 1)),
        "wcat4": np.tile(wcat4, (CORES, 1)),
        "bias": np.tile(bias, (CORES, 1, 1)),
        "bias4": np.tile(bias4, (CORES, 1)),
    }
    dev_in = []
    for name in ex['in_names']:
        if name in ex['const_dev']:
            dev_in.append(ex['const_dev'][name])
        else:
            dev_in.append(jax.device_put(host_in[name], ex['sharding']))
    outs = ex['run'](*dev_in, *ex['dev_zeros'])
    _cache['last_dev_in'] = dev_in
    out_idx = ex['out_names'].index("out")
    out = np.asarray(outs[out_idx]).reshape(CORES, NPC, OUT).reshape(CORES * NPC, OUT)
    return out[:N].astype(np.float32)


def bench_exec(iters=24, warmup=2):
    """Amortized per-execution device time (s) via pipelined dispatch slope.

    Requires a prior kernel() call (persistent executable + device inputs).
    """
    import time
    ex = _cache['exec']
    jax = ex['jax']
    dev_in = _cache['last_dev_in']

    def run_m(m):
        t0 = time.time()
        outs = [ex['run'](*dev_in, *ex['dev_zeros']) for _ in range(m)]
        jax.block_until_ready(outs)
        return time.time() - t0

    for _ in range(warmup):
        run_m(1)
    m1, m2 = 4, iters
    t1 = min(run_m(m1), run_m(m1))
    t2 = min(run_m(m2), run_m(m2))
    slope = (t2 - t1) / (m2 - m1)
    single = min(run_m(1), run_m(1))
    return slope, single
